# revision 1
# baseline (speedup 1.0000x reference)
"""BinaryBasicBlock TRN2 kernel: 8-core batch-parallel, raw Bass.

Reference computation (per core: 8 images, C=64, 56x56):
  y1   = conv3x3(x, sign(w1))            # exact: x = fp16(x) + fp16(residual)
  bin1 = sign((y1 - mu1) * rsqrt(var1+eps) * g1 + b1)   # global batch stats
  y2   = conv3x3(bin1, sign(w2))         # exact (+-1 x +-1 in bf16)
  out  = sign((y2 - mu2) * rsqrt(var2+eps) * g2 + b2 + x)

Batch stats are exact: per-core (sum, sumsq) partials are AllReduced across
the 8 cores mid-kernel.

Layout: channels on partitions, 2 images per 128 partitions (top/bottom
halves), 4 "slots" of [128, 58, 58] padded images per core. Convs run as
9-tap matmul accumulation with all four 64x64 PE quadrants streaming four
different images concurrently; weights are loaded once per (tap, tile) and
reused across both conv1 parts and both PSUM subchunks (LDW amortization).

Toolchain constraints honored: raw Bass only (Tile broken with this walrus),
max one semaphore wait per instruction, single PSUM reader engine per bank,
drain-backed semaphore increments on every cross-engine RAW edge, explicit
DVE drains between dependent vector ops.
"""
import numpy as np
import ml_dtypes
import concourse.bass as bass
import concourse.mybir as mybir
from concourse import bass_utils
from contextlib import ExitStack

F32 = mybir.dt.float32
BF16 = mybir.dt.bfloat16
F16 = mybir.dt.float16
AF = mybir.ActivationFunctionType
ALU = mybir.AluOpType

N_CORES = 8
N, C, H, W = 64, 64, 56, 56
IMGS = N // N_CORES          # 8 images per core
SLOTS = IMGS // 2            # 4 slots (2 images per slot)
QG = SLOTS // 2              # 2 quadgroups (4 images each)
HP = H + 2                   # 58 padded
CHROWS = 8                   # output rows per 448-subchunk
CHUNK = CHROWS * W           # 448
NCH = H // CHROWS            # 7 subchunks per image
SUPERS = [(0, 2), (2, 4), (4, 6), (6, 7)]   # subchunk ranges per super-iter
NSUP = len(SUPERS)           # 4 super-iters per quadgroup
ITERS = QG * NSUP            # 8 super-iters per conv
PERIMG = H * W               # 3136
YCOLS = SLOTS * PERIMG       # 12544
N_TOT = float(N * H * W)     # global batch-stat count
EPS = 1e-5
NF = SLOTS * NSUP            # 16 final-stage iterations (per-slot supers)

DEBUG = False
CC_STUB = False   # replace AllReduce with a local DMA (for TimelineSim)


def build_bass():
    nc = bass.Bass(trn_type="TRN2", target_bir_lowering=False, debug=False,
                   num_devices=N_CORES)

    d_xhi = nc.dram_tensor("xhi", [128, SLOTS, HP, HP], F16, kind="ExternalInput")
    d_xlo = nc.dram_tensor("xlo", [128, SLOTS, HP, HP], F16, kind="ExternalInput")
    d_wf16 = nc.dram_tensor("wf16", [128, 576], F16, kind="ExternalInput")
    d_wbf = nc.dram_tensor("wbf", [128, 576], BF16, kind="ExternalInput")
    d_consts = nc.dram_tensor("consts", [128, 8], F32, kind="ExternalInput")
    d_out = nc.dram_tensor("outp", [128, YCOLS], BF16, kind="ExternalOutput")
    db1_in = nc.dram_tensor("db1_in", [64, 2], F32)
    db1_out = nc.dram_tensor("db1_out", [64, 2], F32, addr_space="Shared")
    db2_in = nc.dram_tensor("db2_in", [64, 2], F32)
    db2_out = nc.dram_tensor("db2_out", [64, 2], F32, addr_space="Shared")
    if DEBUG:
        d_g1 = nc.dram_tensor("dbg_g1", [128, 8], F32, kind="ExternalOutput")
        d_g2 = nc.dram_tensor("dbg_g2", [128, 8], F32, kind="ExternalOutput")
        d_b1 = nc.dram_tensor("dbg_b1", [128, HP * HP], BF16, kind="ExternalOutput")

    es = ExitStack()
    def sb(name, shape, dt):
        return es.enter_context(nc.sbuf_tensor(name, shape, dt))
    def ps(name, shape, dt):
        return es.enter_context(nc.psum_tensor(name, shape, dt))
    def sem(name):
        return es.enter_context(nc.semaphore(name))

    xhi = sb("xhi_t", [128, SLOTS, HP, HP], F16)
    xlo = sb("xlo_t", [128, SLOTS, HP, HP], F16)
    wf16 = sb("wf16_t", [128, 576], F16)
    wbf = sb("wbf_t", [128, 576], BF16)
    consts = sb("consts_t", [128, 8], F32)
    bin1 = sb("bin1_t", [128, SLOTS, HP, HP], BF16)
    y1 = sb("y1_t", [128, YCOLS], F32)
    # y2 (fp16) and the output (bf16) live in y1's bytes (dead by then)
    y2v = y1[:].bitcast(F16)      # [128, 25088] f16 ; cols 0..12543 used
    outv = y1[:].bitcast(BF16)    # [128, 25088] bf16; cols 12544..25087 used
    OUTOFF = YCOLS
    NPART = 2 * ITERS            # partial columns per conv (X and Y per super)
    ps1 = sb("ps1", [128, NPART], F32)
    pq1 = sb("pq1", [128, NPART], F32)
    ps2 = sb("ps2", [128, NPART], F32)
    pq2 = sb("pq2", [128, NPART], F32)
    stats1 = sb("stats1", [128, 8], F32)
    stats2 = sb("stats2", [128, 8], F32)
    glob1 = sb("glob1", [128, 8], F32)
    glob2 = sb("glob2", [128, 8], F32)
    scr = [sb(f"scr{i}", [128, 2 * CHUNK], F32) for i in range(2)]
    ub = [sb(f"ub{i}", [128, 2 * CHUNK], F32) for i in range(2)]
    vb = [sb(f"vb{i}", [128, 2 * CHUNK], F32) for i in range(2)]
    wbuf = [sb(f"wb{i}", [128, 2 * CHUNK], F32) for i in range(2)]
    # PSUM: 2 sets x (X, Y) tensors of 2 banks each = 8 banks
    pbX = [ps(f"pbX{i}", [128, 1024], F32) for i in range(2)]
    pbY = [ps(f"pbY{i}", [128, 1024], F32) for i in range(2)]

    dsem = sem("dsem")
    s_pe1 = sem("s_pe1"); s_ev1 = sem("s_ev1")
    s_pe2 = sem("s_pe2"); s_ev2 = sem("s_ev2")
    s_sg1 = sem("s_sg1"); s_ms = sem("s_ms")
    s_st1 = sem("s_st1"); s_st2 = sem("s_st2"); s_acst = sem("s_acst")
    s_cc = sem("s_cc")
    s_fu = sem("s_fu"); s_fv = sem("s_fv"); s_fs = sem("s_fs")

    CCV = 16 if CC_STUB else 1
    # dsem milestones: loads = xhi01,xlo01,wf16,wbf | xhi23,xlo23,consts
    D_QG0 = 4 * 16
    D_QG1 = 7 * 16
    D_FOLD1 = 8 * 16
    D_B1OUT = 9 * 16
    D_B1IN = 11 * 16     # both halves of the allreduce result loaded
    D_FOLD2 = 12 * 16
    D_B2OUT = 13 * 16
    D_B2IN = 15 * 16

    def ycol(slot, c):
        return slot * PERIMG + c * CHUNK

    # final-stage iteration table: (slot, sub0, nsub)
    FINALS = [(s, c0, c1 - c0) for s in range(SLOTS) for (c0, c1) in SUPERS]

    with nc.Block() as block:

        @block.sync
        def _(sync):
            sync.dma_start(xhi[:, 0:2], d_xhi[:, 0:2]).then_inc(dsem, 16)
            sync.dma_start(xlo[:, 0:2], d_xlo[:, 0:2]).then_inc(dsem, 16)
            sync.dma_start(wf16[:], d_wf16[:]).then_inc(dsem, 16)
            sync.dma_start(wbf[:], d_wbf[:]).then_inc(dsem, 16)
            sync.dma_start(xhi[:, 2:4], d_xhi[:, 2:4]).then_inc(dsem, 16)
            sync.dma_start(xlo[:, 2:4], d_xlo[:, 2:4]).then_inc(dsem, 16)
            sync.dma_start(consts[:], d_consts[:]).then_inc(dsem, 16)
            # stats1 chain
            sync.wait_ge(s_st1, 1)
            sync.dma_start(stats1[0:64, 2:4], stats1[64:128, 0:2]).then_inc(dsem, 16)
            sync.wait_ge(s_st1, 2)
            sync.dma_start(db1_in[:], stats1[0:64, 4:6]).then_inc(dsem, 16)
            sync.wait_ge(s_cc, CCV)
            sync.dma_start(glob1[0:64, 0:2], db1_out[:]).then_inc(dsem, 16)
            sync.dma_start(glob1[64:128, 0:2], db1_out[:]).then_inc(dsem, 16)
            # stats2 chain
            sync.wait_ge(s_st2, 1)
            sync.dma_start(stats2[0:64, 2:4], stats2[64:128, 0:2]).then_inc(dsem, 16)
            sync.wait_ge(s_st2, 2)
            sync.dma_start(db2_in[:], stats2[0:64, 4:6]).then_inc(dsem, 16)
            sync.wait_ge(s_cc, 2 * CCV)
            sync.dma_start(glob2[0:64, 0:2], db2_out[:]).then_inc(dsem, 16)
            sync.dma_start(glob2[64:128, 0:2], db2_out[:]).then_inc(dsem, 16)
            # output stores (per half)
            sync.wait_ge(s_fs, NF // 2)
            sync.dma_start(d_out[:, 0 : YCOLS // 2],
                           outv[:, OUTOFF : OUTOFF + YCOLS // 2]).then_inc(dsem, 16)
            sync.wait_ge(s_fs, NF)
            sync.dma_start(d_out[:, YCOLS // 2 : YCOLS],
                           outv[:, OUTOFF + YCOLS // 2 : OUTOFF + YCOLS]).then_inc(
                               dsem, 16)
            if DEBUG:
                sync.dma_start(d_g1[:], glob1[:]).then_inc(dsem, 16)
                sync.dma_start(d_g2[:], glob2[:]).then_inc(dsem, 16)
                sync.dma_start(d_b1[:], bin1[:, 0]).then_inc(dsem, 16)

        @block.tensor
        def _(tensor):
            def conv(pe_sem, ev_sem, parts, wt, pre_wait, pre_vals, dwaits):
                # parts: list of rhs tensors sharing the same weights per tap
                it = 0
                for q in range(QG):
                    if dwaits is not None:
                        tensor.wait_ge(dsem, dwaits[q])
                    if pre_wait is not None:
                        tensor.wait_ge(pre_wait, pre_vals[q])
                    for (c0, c1) in SUPERS:
                        nsub = c1 - c0
                        if it >= 2:
                            tensor.wait_ge(ev_sem, it - 1)
                        pX = pbX[it % 2]
                        pY = pbY[it % 2]
                        quads = [
                            ((0, 0), slice(0, 64), 2 * q, pX, slice(0, 64)),
                            ((64, 0), slice(64, 128), 2 * q, pY, slice(0, 64)),
                            ((0, 64), slice(0, 64), 2 * q + 1, pX, slice(64, 128)),
                            ((64, 64), slice(64, 128), 2 * q + 1, pY,
                             slice(64, 128)),
                        ]
                        for tap in range(9):
                            kh, kw = tap // 3, tap % 3
                            wcol = tap * 64
                            for tp, rows, _, _, _ in quads:
                                nc.tensor.ldweights(wt[rows, wcol : wcol + 64],
                                                    tile_position=tp)
                            for ip, rhs_t in enumerate(parts):
                                for tp, rows, dslot, pdst, phalf in quads:
                                    for s in range(nsub):
                                        c = c0 + s
                                        first = ip == 0 and tap == 0
                                        last = ip == len(parts) - 1 and tap == 8
                                        rap = rhs_t[rows, dslot,
                                                    c * CHROWS + kh :
                                                    c * CHROWS + kh + CHROWS,
                                                    kw : kw + W]
                                        nc.tensor.matmul(
                                            pdst[phalf, s * 512 : s * 512 + CHUNK],
                                            wt[rows, wcol : wcol + 64], rap,
                                            start=first, stop=last,
                                            tile_position=tp,
                                            skip_group_check=True)
                        tensor.drain().then_inc(pe_sem, 1)
                        it += 1

            conv(s_pe1, s_ev1, [xhi, xlo], wf16, None, None, (D_QG0, D_QG1))
            conv(s_pe2, s_ev2, [bin1], wbf, s_sg1, (2, 4), None)

        @block.scalar
        def _(scalar):
            def evacs(pe_sem, ev_sem, dest, pstats):
                it = 0
                for q in range(QG):
                    for (c0, c1) in SUPERS:
                        nsub = c1 - c0
                        scalar.wait_ge(pe_sem, it + 1)
                        pX = pbX[it % 2]
                        pY = pbY[it % 2]
                        for half, slot, pt in ((0, 2 * q, pX), (1, 2 * q + 1, pY)):
                            src = pt[:, 0 : nsub * 512].rearrange(
                                "p (s k) -> p s k", s=nsub)[:, :, 0:CHUNK]
                            nc.scalar.activation(
                                dest[:, ycol(slot, c0) :
                                     ycol(slot, c0) + nsub * CHUNK],
                                src, AF.Copy,
                                accum_out=pstats[:, 2 * it + half :
                                                 2 * it + half + 1])
                        scalar.drain().then_inc(ev_sem, 1)
                        it += 1

            evacs(s_pe1, s_ev1, y1, ps1)
            # stats1: sqrt step
            scalar.wait_ge(s_st1, 3)
            nc.scalar.activation(glob1[:, 4:5], glob1[:, 5:6], AF.Sqrt)
            scalar.drain().then_inc(s_acst, 1)
            # bin1 = Sign(y1 * a1 + b1) into padded slots (borders pre-zeroed)
            scalar.wait_ge(s_ms, SLOTS + 1)
            scalar.wait_ge(s_st1, 4)
            for s in range(SLOTS):
                nc.scalar.activation(
                    bin1[:, s, 1 : 1 + H, 1 : 1 + W],
                    y1[:, s * PERIMG : (s + 1) * PERIMG],
                    AF.Sign, bias=glob1[:, 7:8], scale=glob1[:, 6:7],
                )
                scalar.drain().then_inc(s_sg1, 1)
            evacs(s_pe2, s_ev2, y2v, ps2)
            # stats2 sqrt
            scalar.wait_ge(s_st2, 3)
            nc.scalar.activation(glob2[:, 4:5], glob2[:, 5:6], AF.Sqrt)
            scalar.drain().then_inc(s_acst, 2)
            # final: u = y2 * a2 ; sign2 = Sign(w + b2)
            scalar.wait_ge(s_st2, 4)

            def sign2(jj):
                sl, c0, nsub = FINALS[jj]
                nc.scalar.activation(
                    outv[:, OUTOFF + ycol(sl, c0) :
                         OUTOFF + ycol(sl, c0) + nsub * CHUNK],
                    wbuf[jj % 2][:, 0 : nsub * CHUNK], AF.Sign,
                    bias=glob2[:, 7:8])
                scalar.drain().then_inc(s_fs, 1)

            for j in range(NF):
                scalar.wait_ge(s_fv, j + 1)
                sign2(j)

        @block.vector
        def _(vector):
            def sumsqs(ev_sem, srcv, pstats):
                it = 0
                for q in range(QG):
                    for (c0, c1) in SUPERS:
                        nsub = c1 - c0
                        vector.wait_ge(ev_sem, it + 1)
                        for half, slot in ((0, 2 * q), (1, 2 * q + 1)):
                            yc = srcv[:, ycol(slot, c0) :
                                      ycol(slot, c0) + nsub * CHUNK]
                            nc.vector.scalar_tensor_tensor(
                                out=scr[it % 2][:, 0 : nsub * CHUNK], in0=yc,
                                scalar=1.0, in1=yc,
                                op0=ALU.mult, op1=ALU.mult,
                                accum_out=pstats[:, 2 * it + half :
                                                 2 * it + half + 1])
                        it += 1

            def stats(pstats_s, pstats_q, st, dsem_fold, dsem_in, acst_v,
                      statst, g, which):
                nc.vector.drain()
                nc.vector.reduce_sum(statst[:, 0:1], pstats_s[:],
                                     axis=mybir.AxisListType.X)
                nc.vector.reduce_sum(statst[:, 1:2], pstats_q[:],
                                     axis=mybir.AxisListType.X)
                nc.vector.drain().then_inc(st, 1)
                vector.wait_ge(dsem, dsem_fold)
                nc.vector.tensor_tensor(out=statst[0:64, 4:6],
                                        in0=statst[0:64, 0:2],
                                        in1=statst[0:64, 2:4], op=ALU.add)
                nc.vector.drain().then_inc(st, 1)
                vector.wait_ge(dsem, dsem_in)
                nc.vector.tensor_scalar_mul(g[:, 2:3], g[:, 0:1], 1.0 / N_TOT)
                nc.vector.tensor_scalar_mul(g[:, 3:4], g[:, 1:2], 1.0 / N_TOT)
                nc.vector.drain()
                nc.vector.tensor_tensor(out=g[:, 4:5], in0=g[:, 2:3],
                                        in1=g[:, 2:3], op=ALU.mult)
                nc.vector.drain()
                nc.vector.tensor_tensor(out=g[:, 5:6], in0=g[:, 3:4],
                                        in1=g[:, 4:5], op=ALU.subtract)
                nc.vector.drain()
                nc.vector.tensor_scalar_add(g[:, 5:6], g[:, 5:6], EPS)
                nc.vector.drain().then_inc(st, 1)
                vector.wait_ge(s_acst, acst_v)
                gcol, bcol = 2 * which, 2 * which + 1
                nc.vector.reciprocal(g[:, 3:4], g[:, 4:5])
                nc.vector.drain()
                nc.vector.tensor_tensor(out=g[:, 6:7], in0=g[:, 3:4],
                                        in1=consts[:, gcol : gcol + 1],
                                        op=ALU.mult)
                nc.vector.drain()
                nc.vector.tensor_tensor(out=g[:, 4:5], in0=g[:, 2:3],
                                        in1=g[:, 6:7], op=ALU.mult)
                nc.vector.drain()
                nc.vector.tensor_tensor(out=g[:, 7:8],
                                        in0=consts[:, bcol : bcol + 1],
                                        in1=g[:, 4:5], op=ALU.subtract)
                nc.vector.drain().then_inc(st, 1)

            sumsqs(s_ev1, y1, pq1)
            stats(ps1, pq1, s_st1, D_FOLD1, D_B1IN, 1, stats1, glob1, 0)
            sumsqs(s_ev2, y2v, pq2)
            stats(ps2, pq2, s_st2, D_FOLD2, D_B2IN, 2, stats2, glob2, 1)
            # final v = (y2 * a2) + xhi ; w = v + xlo
            for j in range(NF):
                sl, c0, nsub = FINALS[j]
                r0 = 1 + c0 * CHROWS
                nr = nsub * CHROWS
                nc.vector.scalar_tensor_tensor(
                    out=vb[j % 2][:, 0 : nsub * CHUNK],
                    in0=y2v[:, ycol(sl, c0) : ycol(sl, c0) + nsub * CHUNK],
                    scalar=glob2[:, 6:7],
                    in1=xhi[:, sl, r0 : r0 + nr, 1 : 1 + W],
                    op0=ALU.mult, op1=ALU.add)
                nc.vector.drain()
                if j >= 2:
                    vector.wait_ge(s_fs, j - 1)
                nc.vector.scalar_tensor_tensor(
                    out=wbuf[j % 2][:, 0 : nsub * CHUNK],
                    in0=vb[j % 2][:, 0 : nsub * CHUNK], scalar=0.0,
                    in1=xlo[:, sl, r0 : r0 + nr, 1 : 1 + W],
                    op0=ALU.add, op1=ALU.add)
                nc.vector.drain().then_inc(s_fv, 1)

        @block.gpsimd
        def _(gpsimd):
            for s in range(SLOTS):
                nc.gpsimd.memset(bin1[:, s], 0).then_inc(s_ms, 1)
            gpsimd.drain().then_inc(s_ms, 1)
            gpsimd.wait_ge(dsem, D_B1OUT)
            if CC_STUB:
                nc.gpsimd.dma_start(db1_out[:], db1_in[:]).then_inc(s_cc, 16)
            else:
                nc.gpsimd.collective_compute(
                    "AllReduce", ALU.add, replica_groups=[list(range(N_CORES))],
                    ins=[db1_in[:]], outs=[db1_out[:]]).then_inc(s_cc, 1)
            gpsimd.wait_ge(dsem, D_B2OUT)
            if CC_STUB:
                nc.gpsimd.dma_start(db2_out[:], db2_in[:]).then_inc(s_cc, 16)
            else:
                nc.gpsimd.collective_compute(
                    "AllReduce", ALU.add, replica_groups=[list(range(N_CORES))],
                    ins=[db2_in[:]], outs=[db2_out[:]]).then_inc(s_cc, 1)

    return nc


_CACHE = {}


def _get_nc():
    if "nc" not in _CACHE:
        _CACHE["nc"] = build_bass()
    return _CACHE["nc"]


def kernel(x, w1, gamma1, beta1, w2, gamma2, beta2):
    x = np.asarray(x, np.float32)
    w1 = np.asarray(w1, np.float32)
    w2 = np.asarray(w2, np.float32)
    gamma1 = np.asarray(gamma1, np.float32)
    beta1 = np.asarray(beta1, np.float32)
    gamma2 = np.asarray(gamma2, np.float32)
    beta2 = np.asarray(beta2, np.float32)

    # binarized weights, [tap, cin, cout] -> [cin, tap*cout], rows duplicated
    def wprep(w):
        wb = np.where(w >= 0, 1.0, -1.0).astype(np.float32)  # [o, i, kh, kw]
        wt = wb.transpose(1, 2, 3, 0).reshape(64, 9, 64).reshape(64, 576)
        return np.concatenate([wt, wt], axis=0)  # [128, 576]

    wf16_np = wprep(w1).astype(np.float16)
    wbf_np = wprep(w2).astype(ml_dtypes.bfloat16)

    consts_np = np.zeros((128, 8), np.float32)
    for col, v in enumerate([gamma1, beta1, gamma2, beta2]):
        consts_np[0:64, col] = v
        consts_np[64:128, col] = v

    in_maps = []
    for k in range(N_CORES):
        xc = x[IMGS * k : IMGS * (k + 1)]            # [8, 64, 56, 56]
        xp = np.zeros((IMGS, C, HP, HP), np.float32)
        xp[:, :, 1 : 1 + H, 1 : 1 + W] = xc
        arr = xp.reshape(SLOTS, 2, C, HP, HP).transpose(1, 2, 0, 3, 4)
        arr = np.ascontiguousarray(arr).reshape(128, SLOTS, HP, HP)
        ahi = arr.astype(np.float16)
        alo = (arr - ahi.astype(np.float32)).astype(np.float16)
        in_maps.append({
            "xhi": ahi, "xlo": alo, "wf16": wf16_np, "wbf": wbf_np,
            "consts": consts_np,
        })

    nc = _get_nc()
    res = bass_utils.run_bass_kernel_spmd(nc, in_maps, core_ids=list(range(N_CORES)))

    out = np.empty((N, C, H, W), np.float32)
    for k in range(N_CORES):
        o = np.asarray(res.results[k]["outp"]).astype(np.float32)  # [128, 12544]
        o = o.reshape(2, C, SLOTS, NCH, CHROWS, W).transpose(2, 0, 1, 3, 4, 5)
        out[IMGS * k : IMGS * (k + 1)] = o.reshape(IMGS, C, H, W)
    return out


if __name__ == "__main__":
    rng = np.random.default_rng(0)
    xs = rng.standard_normal((N, C, H, W)).astype(np.float32)
    w1s = (rng.standard_normal((C, C, 3, 3)) * 0.1).astype(np.float32)
    w2s = (rng.standard_normal((C, C, 3, 3)) * 0.1).astype(np.float32)
    ones = np.ones(C, np.float32)
    zeros = np.zeros(C, np.float32)
    r = kernel(x=xs, w1=w1s, gamma1=ones, beta1=zeros, w2=w2s, gamma2=ones,
               beta2=zeros)
    print("ran, out uniq:", np.unique(r))



# revision 39
# speedup vs baseline: 1.1546x; 1.1546x over previous
"""BinaryBasicBlock TRN2 kernel: 8-core batch-parallel, raw Bass.

Reference computation (per core: 8 images, C=64, 56x56):
  y1   = conv3x3(x, sign(w1))            # exact: x = fp16(x) + fp16(residual)
  bin1 = sign((y1 - mu1) * rsqrt(var1+eps) * g1 + b1)   # global batch stats
  y2   = conv3x3(bin1, sign(w2))         # exact
  out  = sign((y2 - mu2) * rsqrt(var2+eps) * g2 + b2 + x)

Batch stats are exact: per-core (sum, sumsq) partials are AllReduced across
the 8 cores mid-kernel.

v2 speedups over the baseline:
  - conv2 runs in fp8e4 with perf_mode=DoubleRow: bin1 is stored as
    {0,1} (0.5 at the padding halo) so +-1 inputs become exact fp8; the
    0/1 offset is folded into per-channel scalars via S_o = sum(sign(w2))
    (y2 = 2*y2' - S).  Taps pair along kh (pair step 64B, %16-aligned).
  - bin1 row pitch is 64 so a conv2 matmul streams one contiguous
    512-element window (8 rows x 64); the 8 junk columns per row are
    skipped at PSUM evacuation.
  - sign1 (bin1 = is_ge(a1*y1, -b1)) runs on the otherwise-idle GPSIMD
    engine, freeing ACT/DVE in the conv2 phase.
  - PSUM evacuation is split: ACT always reads the pbX banks, DVE always
    reads the pbY banks (one PSUM reader engine per bank).
  - sumsq for conv2 stats and both final residual passes run as all-f16
    tensor_scalar_ptr ops on DVE (4x DVE perf mode).

Toolchain constraints honored: raw Bass only, max one semaphore wait per
instruction, single PSUM reader engine per bank, drain-backed semaphore
increments on every cross-engine RAW edge, explicit DVE drains between
dependent vector ops.
"""
import numpy as np
import ml_dtypes
import concourse.bass as bass
import concourse.mybir as mybir
from concourse.ap import AP
from concourse import bass_utils
from contextlib import ExitStack

F32 = mybir.dt.float32
BF16 = mybir.dt.bfloat16
F16 = mybir.dt.float16
FP8 = mybir.dt.float8e4
AF = mybir.ActivationFunctionType
ALU = mybir.AluOpType
DR = mybir.MatmulPerfMode.DoubleRow

N_CORES = 8
N, C, H, W = 64, 64, 56, 56
IMGS = N // N_CORES          # 8 images per core
SLOTS = IMGS // 2            # 4 slots (2 images per slot)
QG = SLOTS // 2              # 2 quadgroups (4 images each)
HP = H + 2                   # 58 padded
BROWS = H + 3                # 59 rows in the fp8 bin1 (1 extra guard row)
BP = 64                      # bin1 row pitch
CHROWS = 8                   # output rows per 448-subchunk
CHUNK = CHROWS * W           # 448
BCHUNK = CHROWS * BP         # 512 (conv2 psum cols per subchunk)
NCH = H // CHROWS            # 7 subchunks per image
SUPERS = [(0, 2), (2, 4), (4, 6), (6, 7)]   # subchunk ranges per super-iter
NSUP = len(SUPERS)           # 4 super-iters per quadgroup
ITERS = QG * NSUP            # 8 super-iters per conv
PERIMG = H * W               # 3136
YCOLS = SLOTS * PERIMG       # 12544
N_TOT = float(N * H * W)     # global batch-stat count
EPS = 1e-5
NF = SLOTS * NSUP            # 16 final-stage iterations (per-slot supers)

# conv2 DoubleRow tap groups: plane A at (khA, kwA), plane B at (khA+1, kwA).
# zeroA marks groups whose A-plane weights are zero (kh=2 taps ride alone).
C2GROUPS = [
    (0, 0, False),   # taps (0,0)+(1,0)
    (0, 1, False),   # taps (0,1)+(1,1)
    (0, 2, False),   # taps (0,2)+(1,2)
    (1, 0, True),    # zero + tap (2,0)
    (1, 1, True),    # zero + tap (2,1)
    (1, 2, True),    # zero + tap (2,2)
]
NG2 = len(C2GROUPS)
NIT2 = SLOTS * NSUP          # 16 conv2 iterations (slot-major, full-width)

# y1/bin1/y2 slot layout after conv1's quad permutation: slot 2q holds
# images (4q, 4q+2) on its partition halves, slot 2q+1 holds (4q+1, 4q+3).
IMG_OF = {}
for _q in range(QG):
    IMG_OF[2 * _q] = (4 * _q, 4 * _q + 2)
    IMG_OF[2 * _q + 1] = (4 * _q + 1, 4 * _q + 3)

DEBUG = False
CC_STUB = False   # replace AllReduce with a local DMA (for TimelineSim)


def build_bass():
    nc = bass.Bass(trn_type="TRN2", target_bir_lowering=False, debug=False,
                   num_devices=N_CORES)

    d_xhi = nc.dram_tensor("xhi", [128, SLOTS, HP, HP], F16, kind="ExternalInput")
    d_xlo = nc.dram_tensor("xlo", [128, SLOTS, HP, HP], F16, kind="ExternalInput")
    d_xphi = nc.dram_tensor("xphi", [128, YCOLS], F16, kind="ExternalInput")
    d_xplo = nc.dram_tensor("xplo", [128, YCOLS], F16, kind="ExternalInput")
    d_wf16 = nc.dram_tensor("wf16", [128, 576], F16, kind="ExternalInput")
    d_wfp8 = nc.dram_tensor("wfp8", [128, NG2 * 256], FP8, kind="ExternalInput")
    d_consts = nc.dram_tensor("consts", [128, 8], F32, kind="ExternalInput")
    d_out = nc.dram_tensor("outp", [128, YCOLS], BF16, kind="ExternalOutput")
    db1_in = nc.dram_tensor("db1_in", [64, 2], F32)
    db1_out = nc.dram_tensor("db1_out", [64, 2], F32, addr_space="Shared")
    db2_in = nc.dram_tensor("db2_in", [64, 2], F32)
    db2_out = nc.dram_tensor("db2_out", [64, 2], F32, addr_space="Shared")
    if DEBUG:
        d_g1 = nc.dram_tensor("dbg_g1", [128, 8], F32, kind="ExternalOutput")
        d_g2 = nc.dram_tensor("dbg_g2", [128, 8], F32, kind="ExternalOutput")
        d_y2 = nc.dram_tensor("dbg_y2", [128, YCOLS], F16, kind="ExternalOutput")

    es = ExitStack()
    def sb(name, shape, dt):
        return es.enter_context(nc.sbuf_tensor(name, shape, dt))
    def ps(name, shape, dt):
        return es.enter_context(nc.psum_tensor(name, shape, dt))
    def sem(name):
        return es.enter_context(nc.semaphore(name))

    xhi = sb("xhi_t", [128, SLOTS, HP, HP], F16)
    xlo = sb("xlo_t", [128, SLOTS, HP, HP], F16)
    xphi = sb("xphi_t", [128, YCOLS], F16)
    xplo = sb("xplo_t", [128, YCOLS], F16)
    wf16 = sb("wf16_t", [128, 576], F16)
    wfp8 = sb("wfp8_t", [128, NG2 * 256], FP8)
    consts = sb("consts_t", [128, 8], F32)
    bin1 = sb("bin1_t", [128, SLOTS, BROWS, BP], FP8)
    y1 = sb("y1_t", [128, YCOLS], F32)
    # y2 (fp16) and the output (bf16) live in y1's bytes (dead by then)
    y2v = y1[:].bitcast(F16)      # [128, 25088] f16 ; cols 0..12543 used
    outv = y1[:].bitcast(BF16)    # [128, 25088] bf16; cols 12544..25087 used
    OUTOFF = YCOLS
    sa1 = sb("sa1", [128, ITERS], F32)
    sb1 = sb("sb1", [128, ITERS], F32)
    qq1 = sb("qq1", [128, 2 * ITERS], F32)
    # conv2 evac op counts: ACT handles even iters, DVE odd iters
    EOPA = sum(SUPERS[it % NSUP][1] - SUPERS[it % NSUP][0]
               for it in range(NIT2) if it % 2 == 0)
    EOPB = sum(SUPERS[it % NSUP][1] - SUPERS[it % NSUP][0]
               for it in range(NIT2) if it % 2 == 1)
    sa2 = sb("sa2", [128, EOPA], F32)
    sb2 = sb("sb2", [128, EOPB], F32)
    qq2 = sb("qq2", [128, NIT2], F32)
    stats1 = sb("stats1", [128, 8], F32)
    stats2 = sb("stats2", [128, 8], F32)
    glob1 = sb("glob1", [128, 8], F32)
    glob2 = sb("glob2", [128, 8], F32)
    scr = [sb(f"scr{i}", [128, 2 * CHUNK], F32) for i in range(2)]
    scr16 = [s[:].bitcast(F16) for s in scr]
    vb = [sb(f"vb{i}", [128, 2 * CHUNK], F32) for i in range(2)]
    wbuf = [sb(f"wb{i}", [128, 2 * CHUNK], F32) for i in range(2)]
    # PSUM: 2 sets x (X, Y) tensors of 2 banks each = 8 banks
    pbX = [ps(f"pbX{i}", [128, 1024], F32) for i in range(2)]
    pbY = [ps(f"pbY{i}", [128, 1024], F32) for i in range(2)]

    dsem = sem("dsem")
    s_pe1 = sem("s_pe1"); s_pe2 = sem("s_pe2")
    s_eA1 = sem("s_eA1"); s_eB1 = sem("s_eB1")
    s_eA2 = sem("s_eA2"); s_eB2 = sem("s_eB2")
    s_sq1 = sem("s_sq1"); s_sq2 = sem("s_sq2")
    s_st1 = sem("s_st1"); s_st2 = sem("s_st2"); s_acst = sem("s_acst")
    s_m1 = sem("s_m1")
    s_sg1 = sem("s_sg1")
    s_cc = sem("s_cc")
    s_fv = sem("s_fv"); s_fs = sem("s_fs")

    CCV = 16 if CC_STUB else 1
    # dsem milestones (each DMA increments by 16)
    D_QG0 = 4 * 16      # xhi01, xlo01, wf16, wfp8
    D_QG1 = 7 * 16      # xhi23, xlo23, consts
    D_XP = 9 * 16       # xphi, xplo
    D_FOLD1 = 10 * 16
    D_B1DBIN = 11 * 16
    D_G1 = 13 * 16      # both halves of the allreduce-1 result loaded
    D_FOLD2 = 14 * 16
    D_B2DBIN = 15 * 16
    D_G2 = 17 * 16

    def ycol(slot, c):
        return slot * PERIMG + c * CHUNK

    # final-stage iteration table: (slot, sub0, nsub)
    FINALS = [(s, c0, c1 - c0) for s in range(SLOTS) for (c0, c1) in SUPERS]

    with nc.Block() as block:

        @block.sync
        def _(sync):
            sync.dma_start(xhi[:, 0:2], d_xhi[:, 0:2]).then_inc(dsem, 16)
            sync.dma_start(xlo[:, 0:2], d_xlo[:, 0:2]).then_inc(dsem, 16)
            sync.dma_start(wf16[:], d_wf16[:]).then_inc(dsem, 16)
            sync.dma_start(wfp8[:], d_wfp8[:]).then_inc(dsem, 16)
            sync.dma_start(xhi[:, 2:4], d_xhi[:, 2:4]).then_inc(dsem, 16)
            sync.dma_start(xlo[:, 2:4], d_xlo[:, 2:4]).then_inc(dsem, 16)
            sync.dma_start(consts[:], d_consts[:]).then_inc(dsem, 16)
            sync.dma_start(xphi[:], d_xphi[:]).then_inc(dsem, 16)
            sync.dma_start(xplo[:], d_xplo[:]).then_inc(dsem, 16)
            # stats1 chain
            sync.wait_ge(s_st1, 1)
            sync.dma_start(stats1[0:64, 2:4], stats1[64:128, 0:2]).then_inc(dsem, 16)
            sync.wait_ge(s_st1, 2)
            sync.dma_start(db1_in[:], stats1[0:64, 4:6]).then_inc(dsem, 16)
            sync.wait_ge(s_cc, CCV)
            sync.dma_start(glob1[0:64, 0:2], db1_out[:]).then_inc(dsem, 16)
            sync.dma_start(glob1[64:128, 0:2], db1_out[:]).then_inc(dsem, 16)
            # stats2 chain
            sync.wait_ge(s_st2, 1)
            sync.dma_start(stats2[0:64, 2:4], stats2[64:128, 0:2]).then_inc(dsem, 16)
            sync.wait_ge(s_st2, 2)
            sync.dma_start(db2_in[:], stats2[0:64, 4:6]).then_inc(dsem, 16)
            sync.wait_ge(s_cc, 2 * CCV)
            sync.dma_start(glob2[0:64, 0:2], db2_out[:]).then_inc(dsem, 16)
            sync.dma_start(glob2[64:128, 0:2], db2_out[:]).then_inc(dsem, 16)
            # output stores (one per slot)
            for s in range(SLOTS):
                sync.wait_ge(s_fs, NSUP * (s + 1))
                sync.dma_start(
                    d_out[:, s * PERIMG : (s + 1) * PERIMG],
                    outv[:, OUTOFF + s * PERIMG : OUTOFF + (s + 1) * PERIMG]
                ).then_inc(dsem, 16)
            if DEBUG:
                sync.dma_start(d_g1[:], glob1[:]).then_inc(dsem, 16)
                sync.dma_start(d_g2[:], glob2[:]).then_inc(dsem, 16)
                sync.dma_start(d_y2[:], y2v[:, 0:YCOLS]).then_inc(dsem, 16)

        @block.tensor
        def _(tensor):
            # conv1: f16, 9 taps, two passes (hi + lo) into the same psum
            it = 0
            for q in range(QG):
                tensor.wait_ge(dsem, (D_QG0, D_QG1)[q])
                for (c0, c1) in SUPERS:
                    nsub = c1 - c0
                    if it >= 2:
                        tensor.wait_ge(s_eA1, it - 1)
                        tensor.wait_ge(s_eB1, it - 1)
                    pX = pbX[it % 2]
                    pY = pbY[it % 2]
                    quads = [
                        ((0, 0), slice(0, 64), 2 * q, pX, slice(0, 64)),
                        ((64, 0), slice(64, 128), 2 * q, pY, slice(0, 64)),
                        ((0, 64), slice(0, 64), 2 * q + 1, pX, slice(64, 128)),
                        ((64, 64), slice(64, 128), 2 * q + 1, pY,
                         slice(64, 128)),
                    ]
                    for tap in range(9):
                        kh, kw = tap // 3, tap % 3
                        wcol = tap * 64
                        for tp, rows, _, _, _ in quads:
                            nc.tensor.ldweights(wf16[rows, wcol : wcol + 64],
                                                tile_position=tp)
                        for ip, rhs_t in enumerate([xhi, xlo]):
                            for tp, rows, dslot, pdst, phalf in quads:
                                for s in range(nsub):
                                    c = c0 + s
                                    first = ip == 0 and tap == 0
                                    last = ip == 1 and tap == 8
                                    rap = rhs_t[rows, dslot,
                                                c * CHROWS + kh :
                                                c * CHROWS + kh + CHROWS,
                                                kw : kw + W]
                                    nc.tensor.matmul(
                                        pdst[phalf, s * 512 : s * 512 + CHUNK],
                                        wf16[rows, wcol : wcol + 64], rap,
                                        start=first, stop=last,
                                        tile_position=tp,
                                        skip_group_check=True)
                    tensor.drain().then_inc(s_pe1, 1)
                    it += 1

            # conv2: full-width fp8 DoubleRow (block-diagonal weights handle
            # both images of a slot per matmul), slot-major, 16 iterations
            PSUMS = [pbX[0], pbY[0], pbX[1], pbY[1]]
            for it in range(NIT2):
                slot, ci = it // NSUP, it % NSUP
                c0, c1 = SUPERS[ci]
                nsub = c1 - c0
                tensor.wait_ge(s_sg1, it + 1)
                if it >= 4:
                    if it % 2 == 0:
                        tensor.wait_ge(s_eA2, (it - 4) // 2 + 1)
                    else:
                        tensor.wait_ge(s_eB2, (it - 5) // 2 + 1)
                pdst = PSUMS[it % 4]
                v = bin1[:, slot]
                pstride = v.ap[0][0]
                for g, (khA, kwA, _) in enumerate(C2GROUPS):
                    wg = wfp8[:, g * 256 : (g + 1) * 256].rearrange(
                        "p (a b) -> p a b", a=2)
                    nc.tensor.ldweights(wg, perf_mode=DR)
                    for s in range(nsub):
                        c = c0 + s
                        off = v.offset + (c * CHROWS + khA) * BP + kwA
                        rap = AP(tensor=v.tensor, offset=off,
                                 ap=[[pstride, 128], [BP, 2], [1, BCHUNK]])
                        nc.tensor.matmul(
                            pdst[:, s * 512 : (s + 1) * 512],
                            wg, rap,
                            start=(g == 0), stop=(g == NG2 - 1),
                            perf_mode=DR,
                            skip_group_check=True)
                tensor.drain().then_inc(s_pe2, 1)

        @block.scalar
        def _(scalar):
            # conv1 evac: ACT reads pbX (slots 2q), sum accum into sa1
            it = 0
            for q in range(QG):
                for (c0, c1) in SUPERS:
                    nsub = c1 - c0
                    scalar.wait_ge(s_pe1, it + 1)
                    pX = pbX[it % 2]
                    src = pX[:, 0 : nsub * 512].rearrange(
                        "p (s k) -> p s k", s=nsub)[:, :, 0:CHUNK]
                    nc.scalar.activation(
                        y1[:, ycol(2 * q, c0) : ycol(2 * q, c0) + nsub * CHUNK],
                        src, AF.Copy,
                        accum_out=sa1[:, it : it + 1])
                    scalar.drain().then_inc(s_eA1, 1)
                    it += 1
            # stats1: sqrt(var + eps)
            scalar.wait_ge(s_st1, 3)
            nc.scalar.activation(glob1[:, 4:5], glob1[:, 5:6], AF.Sqrt)
            scalar.drain().then_inc(s_acst, 1)
            # conv2 evac: ACT reads the pbX-rotation iters (even its),
            # per-subchunk so the 8 junk cols per row are skipped
            PSUMS_S = [pbX[0], pbY[0], pbX[1], pbY[1]]
            eop = 0
            for it in range(NIT2):
                if it % 2 != 0:
                    continue
                slot, ci = it // NSUP, it % NSUP
                c0, c1 = SUPERS[ci]
                nsub = c1 - c0
                scalar.wait_ge(s_pe2, it + 1)
                pt = PSUMS_S[it % 4][:]
                pstride = pt.ap[0][0]
                for s in range(nsub):
                    src = AP(tensor=pt.tensor, offset=pt.offset + s * 512,
                             ap=[[pstride, 128], [BP, CHROWS], [1, W]])
                    # y2'' = y2' - S/2, so y2 = 2*y2'' exactly (S is even)
                    nc.scalar.activation(
                        y2v[:, ycol(slot, c0 + s) :
                            ycol(slot, c0 + s) + CHUNK],
                        src, AF.Identity, bias=consts[:, 4:5],
                        accum_out=sa2[:, eop : eop + 1])
                    eop += 1
                scalar.drain().then_inc(s_eA2, 1)
            # stats2 sqrt
            scalar.wait_ge(s_st2, 3)
            nc.scalar.activation(glob2[:, 4:5], glob2[:, 5:6], AF.Sqrt)
            scalar.drain().then_inc(s_acst, 2)
            # final: sign2 = Sign(w)  (bias2' already folded into w)
            for j in range(NF):
                sl, c0, nsub = FINALS[j]
                scalar.wait_ge(s_fv, j + 1)
                nc.scalar.activation(
                    outv[:, OUTOFF + ycol(sl, c0) :
                         OUTOFF + ycol(sl, c0) + nsub * CHUNK],
                    wbuf[j % 2][:, 0 : nsub * CHUNK], AF.Sign)
                scalar.drain().then_inc(s_fs, 1)

        @block.vector
        def _(vector):
            # conv1: DVE evacs pbY (slots 2q+1) + sumsq over both slots
            it = 0
            for q in range(QG):
                for (c0, c1) in SUPERS:
                    nsub = c1 - c0
                    vector.wait_ge(s_pe1, it + 1)
                    pY = pbY[it % 2]
                    src = pY[:, 0 : nsub * 512].rearrange(
                        "p (s k) -> p s k", s=nsub)[:, :, 0:CHUNK]
                    nc.vector.tensor_scalar(
                        out=y1[:, ycol(2 * q + 1, c0) :
                               ycol(2 * q + 1, c0) + nsub * CHUNK],
                        in0=src, scalar1=0.0, scalar2=None,
                        op0=ALU.add, op1=ALU.add,
                        accum_out=sb1[:, it : it + 1])
                    nc.vector.drain().then_inc(s_eB1, 1)
                    vector.wait_ge(s_eA1, it + 1)
                    for half, slot in ((0, 2 * q), (1, 2 * q + 1)):
                        yc = y1[:, ycol(slot, c0) :
                                ycol(slot, c0) + nsub * CHUNK]
                        nc.vector.scalar_tensor_tensor(
                            out=scr[it % 2][:, 0 : nsub * CHUNK], in0=yc,
                            scalar=1.0, in1=yc,
                            op0=ALU.mult, op1=ALU.mult,
                            accum_out=qq1[:, 2 * it + half :
                                          2 * it + half + 1])
                    nc.vector.drain().then_inc(s_sq1, 1)
                    it += 1

            # stats1 fold + math: a1 = g1*rsqrt(var+eps), nb1 = m*a1 - b1
            nc.vector.reduce_sum(stats1[:, 6:7], sa1[:], axis=mybir.AxisListType.X)
            nc.vector.reduce_sum(stats1[:, 7:8], sb1[:], axis=mybir.AxisListType.X)
            nc.vector.reduce_sum(stats1[:, 1:2], qq1[:], axis=mybir.AxisListType.X)
            nc.vector.drain()
            nc.vector.tensor_tensor(out=stats1[:, 0:1], in0=stats1[:, 6:7],
                                    in1=stats1[:, 7:8], op=ALU.add)
            nc.vector.drain().then_inc(s_st1, 1)
            vector.wait_ge(dsem, D_FOLD1)
            nc.vector.tensor_tensor(out=stats1[0:64, 4:6],
                                    in0=stats1[0:64, 0:2],
                                    in1=stats1[0:64, 2:4], op=ALU.add)
            nc.vector.drain().then_inc(s_st1, 1)
            vector.wait_ge(dsem, D_G1)
            nc.vector.tensor_scalar_mul(glob1[:, 2:4], glob1[:, 0:2],
                                        1.0 / N_TOT)
            nc.vector.drain()
            nc.vector.tensor_tensor(out=glob1[:, 4:5], in0=glob1[:, 2:3],
                                    in1=glob1[:, 2:3], op=ALU.mult)
            nc.vector.drain()
            nc.vector.tensor_tensor(out=glob1[:, 5:6], in0=glob1[:, 3:4],
                                    in1=glob1[:, 4:5], op=ALU.subtract)
            nc.vector.drain()
            nc.vector.tensor_scalar_add(glob1[:, 5:6], glob1[:, 5:6], EPS)
            nc.vector.drain().then_inc(s_st1, 1)
            vector.wait_ge(s_acst, 1)
            nc.vector.reciprocal(glob1[:, 3:4], glob1[:, 4:5])
            nc.vector.drain()
            nc.vector.tensor_tensor(out=glob1[:, 6:7], in0=glob1[:, 3:4],
                                    in1=consts[:, 0:1], op=ALU.mult)
            nc.vector.drain()
            nc.vector.tensor_tensor(out=glob1[:, 4:5], in0=glob1[:, 2:3],
                                    in1=glob1[:, 6:7], op=ALU.mult)
            nc.vector.drain()
            nc.vector.tensor_tensor(out=glob1[:, 7:8], in0=glob1[:, 4:5],
                                    in1=consts[:, 1:2], op=ALU.subtract)
            nc.vector.drain().then_inc(s_m1, 1)

            # conv2: DVE evacs the pbY-rotation iters (odd its) + f16 sumsq
            # over every iter's fresh y2' columns (scaled by 1/64)
            PSUMS_V = [pbX[0], pbY[0], pbX[1], pbY[1]]
            eop = 0
            for it in range(NIT2):
                slot, ci = it // NSUP, it % NSUP
                c0, c1 = SUPERS[ci]
                nsub = c1 - c0
                if it % 2 == 1:
                    vector.wait_ge(s_pe2, it + 1)
                    pt = PSUMS_V[it % 4][:]
                    pstride = pt.ap[0][0]
                    for s in range(nsub):
                        src = AP(tensor=pt.tensor, offset=pt.offset + s * 512,
                                 ap=[[pstride, 128], [BP, CHROWS], [1, W]])
                        nc.vector.tensor_scalar(
                            out=y2v[:, ycol(slot, c0 + s) :
                                    ycol(slot, c0 + s) + CHUNK],
                            in0=src, scalar1=consts[:, 4:5], scalar2=None,
                            op0=ALU.add, op1=ALU.add,
                            accum_out=sb2[:, eop : eop + 1])
                        eop += 1
                    nc.vector.drain().then_inc(s_eB2, 1)
                else:
                    vector.wait_ge(s_eA2, it // 2 + 1)
                yc = y2v[:, ycol(slot, c0) : ycol(slot, c0) + nsub * CHUNK]
                nc.vector.scalar_tensor_tensor(
                    out=scr16[it % 2][:, 0 : nsub * CHUNK], in0=yc,
                    scalar=1.0 / 64.0, in1=yc,
                    op0=ALU.mult, op1=ALU.mult,
                    accum_out=qq2[:, it : it + 1])
                nc.vector.drain().then_inc(s_sq2, 1)

            # stats2 fold + math: y2 = 2*y2'' exactly, so
            #   m2 = 2*m'' ; var2 = 256*q'' - (2*m'')^2 ; SC = 2*g2*rsqrt(var2+eps)
            nc.vector.reduce_sum(stats2[:, 6:7], sa2[:], axis=mybir.AxisListType.X)
            nc.vector.reduce_sum(stats2[:, 7:8], sb2[:], axis=mybir.AxisListType.X)
            nc.vector.reduce_sum(stats2[:, 1:2], qq2[:], axis=mybir.AxisListType.X)
            nc.vector.drain()
            nc.vector.tensor_tensor(out=stats2[:, 0:1], in0=stats2[:, 6:7],
                                    in1=stats2[:, 7:8], op=ALU.add)
            nc.vector.drain().then_inc(s_st2, 1)
            vector.wait_ge(dsem, D_FOLD2)
            nc.vector.tensor_tensor(out=stats2[0:64, 4:6],
                                    in0=stats2[0:64, 0:2],
                                    in1=stats2[0:64, 2:4], op=ALU.add)
            nc.vector.drain().then_inc(s_st2, 1)
            vector.wait_ge(dsem, D_G2)
            nc.vector.tensor_scalar_mul(glob2[:, 2:4], glob2[:, 0:2],
                                        1.0 / N_TOT)
            nc.vector.drain()
            # col4 = (2*m'')^2 ; col5 = 256*q'' - col4 + eps = var2 + eps
            nc.vector.tensor_scalar_mul(glob2[:, 4:5], glob2[:, 2:3], 2.0)
            nc.vector.drain()
            nc.vector.tensor_tensor(out=glob2[:, 4:5], in0=glob2[:, 4:5],
                                    in1=glob2[:, 4:5], op=ALU.mult)
            nc.vector.drain()
            nc.vector.scalar_tensor_tensor(
                out=glob2[:, 5:6], in0=glob2[:, 3:4], scalar=256.0,
                in1=glob2[:, 4:5], op0=ALU.mult, op1=ALU.subtract)
            nc.vector.drain()
            nc.vector.tensor_scalar_add(glob2[:, 5:6], glob2[:, 5:6], EPS)
            nc.vector.drain().then_inc(s_st2, 1)
            vector.wait_ge(s_acst, 2)
            nc.vector.reciprocal(glob2[:, 3:4], glob2[:, 4:5])
            nc.vector.drain()
            # col6 = A2 = recip * g2 ; col7 = bias2' = beta2 - 2*A2*m'' ;
            # then col6 = SC = 2*A2  (m'' still lives in col2)
            nc.vector.tensor_tensor(out=glob2[:, 6:7], in0=glob2[:, 3:4],
                                    in1=consts[:, 2:3], op=ALU.mult)
            nc.vector.drain()
            nc.vector.tensor_tensor(out=glob2[:, 4:5], in0=glob2[:, 6:7],
                                    in1=glob2[:, 2:3], op=ALU.mult)
            nc.vector.drain()
            nc.vector.tensor_scalar_mul(glob2[:, 4:5], glob2[:, 4:5], 2.0)
            nc.vector.drain()
            nc.vector.tensor_tensor(out=glob2[:, 7:8], in0=consts[:, 3:4],
                                    in1=glob2[:, 4:5], op=ALU.subtract)
            nc.vector.tensor_scalar_mul(glob2[:, 6:7], glob2[:, 6:7], 2.0)
            nc.vector.drain()

            # final (all-f16 STT, 4x DVE mode; sign-safe because y2'' is
            # S-centered and beta2 folds into pass B before the f16 round):
            #   v = SC*y2'' + xphi ; w = (v + beta2) + xplo
            for j in range(NF):
                sl, c0, nsub = FINALS[j]
                cols = slice(ycol(sl, c0), ycol(sl, c0) + nsub * CHUNK)
                nc.vector.scalar_tensor_tensor(
                    out=vb[j % 2][:, 0 : nsub * CHUNK],
                    in0=y2v[:, cols],
                    scalar=glob2[:, 6:7],
                    in1=xphi[:, cols],
                    op0=ALU.mult, op1=ALU.add)
                nc.vector.drain()
                if j >= 2:
                    vector.wait_ge(s_fs, j - 1)
                nc.vector.scalar_tensor_tensor(
                    out=wbuf[j % 2][:, 0 : nsub * CHUNK],
                    in0=vb[j % 2][:, 0 : nsub * CHUNK],
                    scalar=glob2[:, 7:8],
                    in1=xplo[:, cols],
                    op0=ALU.add, op1=ALU.add)
                nc.vector.drain().then_inc(s_fv, 1)

        @block.gpsimd
        def _(gpsimd):
            # bin1 halo = 0.5 (== (0+1)/2, the zero-pad in {0,1} space)
            for s in range(SLOTS):
                nc.gpsimd.memset(bin1[:, s], 0.5)
            gpsimd.drain()
            gpsimd.wait_ge(dsem, D_B1DBIN)
            if CC_STUB:
                nc.gpsimd.dma_start(db1_out[:], db1_in[:]).then_inc(s_cc, 16)
            else:
                nc.gpsimd.collective_compute(
                    "AllReduce", ALU.add, replica_groups=[list(range(N_CORES))],
                    ins=[db1_in[:]], outs=[db1_out[:]]).then_inc(s_cc, 1)
            # sign1: bin1 = (a1*y1 >= -b1) in {0,1}, written as fp8.
            # Slot-major emission matches conv2's iteration order.
            gpsimd.wait_ge(s_m1, 1)
            for slot in range(SLOTS):
                for (c0, c1) in SUPERS:
                    nsub = c1 - c0
                    nc.gpsimd.tensor_scalar(
                        out=bin1[:, slot, 1 + c0 * CHROWS :
                                 1 + c1 * CHROWS, 1 : 1 + W],
                        in0=y1[:, ycol(slot, c0) :
                               ycol(slot, c0) + nsub * CHUNK],
                        scalar1=glob1[:, 6:7], scalar2=glob1[:, 7:8],
                        op0=ALU.mult, op1=ALU.is_ge)
                    gpsimd.drain().then_inc(s_sg1, 1)
            gpsimd.wait_ge(dsem, D_B2DBIN)
            if CC_STUB:
                nc.gpsimd.dma_start(db2_out[:], db2_in[:]).then_inc(s_cc, 16)
            else:
                nc.gpsimd.collective_compute(
                    "AllReduce", ALU.add, replica_groups=[list(range(N_CORES))],
                    ins=[db2_in[:]], outs=[db2_out[:]]).then_inc(s_cc, 1)

    return nc


_CACHE = {}


def _get_nc():
    if "nc" not in _CACHE:
        _CACHE["nc"] = build_bass()
    return _CACHE["nc"]


def kernel(x, w1, gamma1, beta1, w2, gamma2, beta2):
    x = np.asarray(x, np.float32)
    w1 = np.asarray(w1, np.float32)
    w2 = np.asarray(w2, np.float32)
    gamma1 = np.asarray(gamma1, np.float32)
    beta1 = np.asarray(beta1, np.float32)
    gamma2 = np.asarray(gamma2, np.float32)
    beta2 = np.asarray(beta2, np.float32)

    # conv1 weights: [tap, cin, cout] -> [cin, tap*cout], rows duplicated
    wb1 = np.where(w1 >= 0, 1.0, -1.0).astype(np.float32)
    wt1 = wb1.transpose(1, 2, 3, 0).reshape(64, 9, 64).reshape(64, 576)
    wf16_np = np.concatenate([wt1, wt1], axis=0).astype(np.float16)

    # conv2 weights: sign(w2) as fp8, full-width DoubleRow with
    # block-diagonal planes (two images per matmul, two kh-taps per pass)
    wb2 = np.where(w2 >= 0, 1.0, -1.0).astype(np.float32)   # [o, i, kh, kw]
    wtap = wb2.transpose(2, 3, 1, 0)                        # [kh, kw, i, o]
    wfp8_np = np.zeros((128, NG2 * 256), np.float32)
    for g, (khA, kwA, zeroA) in enumerate(C2GROUPS):
        for i in (0, 1):
            if i == 0 and zeroA:
                continue
            blk = np.zeros((128, 128), np.float32)
            blk[0:64, 0:64] = wtap[khA + i, kwA]
            blk[64:128, 64:128] = wtap[khA + i, kwA]
            wfp8_np[:, g * 256 + i * 128 : g * 256 + (i + 1) * 128] = blk
    wfp8_np = wfp8_np.astype(ml_dtypes.float8_e4m3)

    S = wb2.sum(axis=(1, 2, 3))                             # [64] per out-ch
    consts_np = np.zeros((128, 8), np.float32)
    for col, v in enumerate([gamma1, beta1, gamma2, beta2, -0.5 * S]):
        consts_np[0:64, col] = v
        consts_np[64:128, col] = v

    in_maps = []
    for k in range(N_CORES):
        xc = x[IMGS * k : IMGS * (k + 1)]            # [8, 64, 56, 56]
        xp = np.zeros((IMGS, C, HP, HP), np.float32)
        xp[:, :, 1 : 1 + H, 1 : 1 + W] = xc
        arr = xp.reshape(SLOTS, 2, C, HP, HP).transpose(1, 2, 0, 3, 4)
        arr = np.ascontiguousarray(arr).reshape(128, SLOTS, HP, HP)
        ahi = arr.astype(np.float16)
        alo = (arr - ahi.astype(np.float32)).astype(np.float16)
        # second x copy in y1's permuted slot order (interior only) for
        # the final residual add
        xperm = np.empty((2, C, SLOTS, H, W), np.float32)
        for s in range(SLOTS):
            for h in (0, 1):
                xperm[h, :, s] = xc[IMG_OF[s][h]]
        xperm = xperm.reshape(128, YCOLS)
        xphi_np = xperm.astype(np.float16)
        xplo_np = (xperm - xphi_np.astype(np.float32)).astype(np.float16)
        in_maps.append({
            "xhi": ahi, "xlo": alo, "xphi": xphi_np, "xplo": xplo_np,
            "wf16": wf16_np, "wfp8": wfp8_np, "consts": consts_np,
        })

    nc = _get_nc()
    res = bass_utils.run_bass_kernel_spmd(nc, in_maps, core_ids=list(range(N_CORES)))

    out = np.empty((N, C, H, W), np.float32)
    for k in range(N_CORES):
        o = np.asarray(res.results[k]["outp"]).astype(np.float32)  # [128, 12544]
        o = o.reshape(2, C, SLOTS, H, W)
        for s in range(SLOTS):
            for h in (0, 1):
                out[IMGS * k + IMG_OF[s][h]] = o[h, :, s]
    return out


if __name__ == "__main__":
    rng = np.random.default_rng(0)
    xs = rng.standard_normal((N, C, H, W)).astype(np.float32)
    w1s = (rng.standard_normal((C, C, 3, 3)) * 0.1).astype(np.float32)
    w2s = (rng.standard_normal((C, C, 3, 3)) * 0.1).astype(np.float32)
    ones = np.ones(C, np.float32)
    zeros = np.zeros(C, np.float32)
    r = kernel(x=xs, w1=w1s, gamma1=ones, beta1=zeros, w2=w2s, gamma2=ones,
               beta2=zeros)
    print("ran, out uniq:", np.unique(r))


# revision 41
# speedup vs baseline: 1.3078x; 1.1327x over previous
"""BinaryBasicBlock TRN2 kernel: 8-core batch-parallel, raw Bass.

Reference computation (per core: 8 images, C=64, 56x56):
  y1   = conv3x3(x, sign(w1))            # exact: x = fp16(x) + fp16(residual)
  bin1 = sign((y1 - mu1) * rsqrt(var1+eps) * g1 + b1)   # global batch stats
  y2   = conv3x3(bin1, sign(w2))         # exact
  out  = sign((y2 - mu2) * rsqrt(var2+eps) * g2 + b2 + x)

Batch stats are exact: per-core (sum, sumsq) partials are AllReduced across
the 8 cores mid-kernel.

v2 speedups over the baseline:
  - conv2 runs in fp8e4 with perf_mode=DoubleRow: bin1 is stored as
    {0,1} (0.5 at the padding halo) so +-1 inputs become exact fp8; the
    0/1 offset is folded into per-channel scalars via S_o = sum(sign(w2))
    (y2 = 2*y2' - S).  Taps pair along kh (pair step 64B, %16-aligned).
  - bin1 row pitch is 64 so a conv2 matmul streams one contiguous
    512-element window (8 rows x 64); the 8 junk columns per row are
    skipped at PSUM evacuation.
  - sign1 (bin1 = is_ge(a1*y1, -b1)) runs on the otherwise-idle GPSIMD
    engine, freeing ACT/DVE in the conv2 phase.
  - PSUM evacuation is split: ACT always reads the pbX banks, DVE always
    reads the pbY banks (one PSUM reader engine per bank).
  - sumsq for conv2 stats and both final residual passes run as all-f16
    tensor_scalar_ptr ops on DVE (4x DVE perf mode).

Toolchain constraints honored: raw Bass only, max one semaphore wait per
instruction, single PSUM reader engine per bank, drain-backed semaphore
increments on every cross-engine RAW edge, explicit DVE drains between
dependent vector ops.
"""
import numpy as np
import ml_dtypes
import concourse.bass as bass
import concourse.mybir as mybir
from concourse.ap import AP
from concourse import bass_utils
from contextlib import ExitStack

F32 = mybir.dt.float32
BF16 = mybir.dt.bfloat16
F16 = mybir.dt.float16
FP8 = mybir.dt.float8e4
AF = mybir.ActivationFunctionType
ALU = mybir.AluOpType
DR = mybir.MatmulPerfMode.DoubleRow

N_CORES = 8
N, C, H, W = 64, 64, 56, 56
IMGS = N // N_CORES          # 8 images per core
SLOTS = IMGS // 2            # 4 slots (2 images per slot)
QG = SLOTS // 2              # 2 quadgroups (4 images each)
HP = H + 2                   # 58 padded
BROWS = H + 3                # 59 rows in the fp8 bin1 (1 extra guard row)
BP = 64                      # bin1 row pitch
CHROWS = 8                   # output rows per 448-subchunk
CHUNK = CHROWS * W           # 448
BCHUNK = CHROWS * BP         # 512 (conv2 psum cols per subchunk)
NCH = H // CHROWS            # 7 subchunks per image
SUPERS = [(0, 2), (2, 4), (4, 6), (6, 7)]   # subchunk ranges per super-iter
NSUP = len(SUPERS)           # 4 super-iters per quadgroup
ITERS = QG * NSUP            # 8 super-iters per conv
PERIMG = H * W               # 3136
YCOLS = SLOTS * PERIMG       # 12544
N_TOT = float(N * H * W)     # global batch-stat count
EPS = 1e-5
NF = SLOTS * NSUP            # 16 final-stage iterations (per-slot supers)

# conv2 DoubleRow tap groups: plane A at (khA, kwA), plane B at (khA+1, kwA).
# zeroA marks groups whose A-plane weights are zero (kh=2 taps ride alone).
C2GROUPS = [
    (0, 0, False),   # taps (0,0)+(1,0)
    (0, 1, False),   # taps (0,1)+(1,1)
    (0, 2, False),   # taps (0,2)+(1,2)
    (1, 0, True),    # zero + tap (2,0)
    (1, 1, True),    # zero + tap (2,1)
    (1, 2, True),    # zero + tap (2,2)
]
NG2 = len(C2GROUPS)
NIT2 = SLOTS * NSUP          # 16 conv2 iterations (slot-major, full-width)

# y1/bin1/y2 slot layout after conv1's quad permutation: slot 2q holds
# images (4q, 4q+2) on its partition halves, slot 2q+1 holds (4q+1, 4q+3).
IMG_OF = {}
for _q in range(QG):
    IMG_OF[2 * _q] = (4 * _q, 4 * _q + 2)
    IMG_OF[2 * _q + 1] = (4 * _q + 1, 4 * _q + 3)

DEBUG = False
CC_STUB = False   # replace AllReduce with a local DMA (for TimelineSim)


def build_bass():
    nc = bass.Bass(trn_type="TRN2", target_bir_lowering=False, debug=False,
                   num_devices=N_CORES)

    d_xhi = nc.dram_tensor("xhi", [128, SLOTS, HP, HP], F16, kind="ExternalInput")
    d_xlo = nc.dram_tensor("xlo", [128, SLOTS, HP, HP], F16, kind="ExternalInput")
    d_x32 = nc.dram_tensor("x32", [128, YCOLS], F32, kind="ExternalInput")
    d_wf16 = nc.dram_tensor("wf16", [128, 576], F16, kind="ExternalInput")
    d_wfp8 = nc.dram_tensor("wfp8", [128, NG2 * 256], FP8, kind="ExternalInput")
    d_consts = nc.dram_tensor("consts", [128, 8], F32, kind="ExternalInput")
    d_out = nc.dram_tensor("outp", [128, YCOLS], BF16, kind="ExternalOutput")
    db1_in = nc.dram_tensor("db1_in", [128, 2], F32)
    db1_out = nc.dram_tensor("db1_out", [128, 2], F32, addr_space="Shared")
    db2_in = nc.dram_tensor("db2_in", [128, 2], F32)
    db2_out = nc.dram_tensor("db2_out", [128, 2], F32, addr_space="Shared")
    if DEBUG:
        d_g1 = nc.dram_tensor("dbg_g1", [128, 8], F32, kind="ExternalOutput")
        d_g2 = nc.dram_tensor("dbg_g2", [128, 8], F32, kind="ExternalOutput")
        d_y2 = nc.dram_tensor("dbg_y2", [128, YCOLS], F16, kind="ExternalOutput")

    es = ExitStack()
    def sb(name, shape, dt):
        return es.enter_context(nc.sbuf_tensor(name, shape, dt))
    def ps(name, shape, dt):
        return es.enter_context(nc.psum_tensor(name, shape, dt))
    def sem(name):
        return es.enter_context(nc.semaphore(name))

    xhi = sb("xhi_t", [128, SLOTS, HP, HP], F16)
    xlo = sb("xlo_t", [128, SLOTS, HP, HP], F16)
    x32 = sb("x32_t", [128, YCOLS], F32)
    wf16 = sb("wf16_t", [128, 576], F16)
    wfp8 = sb("wfp8_t", [128, NG2 * 256], FP8)
    consts = sb("consts_t", [128, 8], F32)
    bin1 = sb("bin1_t", [128, SLOTS, BROWS, BP], FP8)
    y1 = sb("y1_t", [128, YCOLS], F32)
    # y2 (fp16) and the output (bf16) live in y1's bytes (dead by then)
    y2v = y1[:].bitcast(F16)      # [128, 25088] f16 ; cols 0..12543 used
    outv = y1[:].bitcast(BF16)    # [128, 25088] bf16; cols 12544..25087 used
    OUTOFF = YCOLS
    sa1 = sb("sa1", [128, ITERS], F32)
    sb1 = sb("sb1", [128, ITERS], F32)
    qq1 = sb("qq1", [128, 2 * ITERS], F32)
    # conv2 evac op counts: ACT handles even iters, DVE odd iters
    EOPA = sum(SUPERS[it % NSUP][1] - SUPERS[it % NSUP][0]
               for it in range(NIT2) if it % 2 == 0)
    EOPB = sum(SUPERS[it % NSUP][1] - SUPERS[it % NSUP][0]
               for it in range(NIT2) if it % 2 == 1)
    sa2 = sb("sa2", [128, EOPA], F32)
    sb2 = sb("sb2", [128, EOPB], F32)
    qq2 = sb("qq2", [128, NIT2], F32)
    stats1 = sb("stats1", [128, 8], F32)
    stats2 = sb("stats2", [128, 8], F32)
    glob1 = sb("glob1", [128, 8], F32)
    glob2 = sb("glob2", [128, 8], F32)
    scr = [sb(f"scr{i}", [128, 2 * CHUNK], F32) for i in range(2)]
    scr16 = [s[:].bitcast(F16) for s in scr]
    wbuf = [sb(f"wb{i}", [128, 2 * CHUNK], F32) for i in range(2)]
    # PSUM: 2 sets x (X, Y) tensors of 2 banks each = 8 banks
    pbX = [ps(f"pbX{i}", [128, 1024], F32) for i in range(2)]
    pbY = [ps(f"pbY{i}", [128, 1024], F32) for i in range(2)]

    dsem = sem("dsem")
    s_pe1 = sem("s_pe1"); s_pe2 = sem("s_pe2")
    s_eA1 = sem("s_eA1"); s_eB1 = sem("s_eB1")
    s_eA2 = sem("s_eA2"); s_eB2 = sem("s_eB2")
    s_sq1 = sem("s_sq1"); s_sq2 = sem("s_sq2")
    s_st1 = sem("s_st1"); s_st2 = sem("s_st2"); s_acst = sem("s_acst")
    s_m1 = sem("s_m1")
    s_sg1 = sem("s_sg1")
    s_cc = sem("s_cc")
    s_fv = sem("s_fv"); s_fs = sem("s_fs")

    CCV = 16 if CC_STUB else 1
    # dsem milestones (each DMA increments by 16)
    D_QG0 = 4 * 16      # xhi01, xlo01, wf16, wfp8
    D_QG1 = 7 * 16      # xhi23, xlo23, consts
    D_XP = 9 * 16       # x32 halves
    D_B1DBIN = 10 * 16
    D_G1 = 13 * 16      # allreduce-1 result + swapped halves loaded
    D_B2DBIN = 14 * 16
    D_G2 = 17 * 16

    def ycol(slot, c):
        return slot * PERIMG + c * CHUNK

    # final-stage iteration table: (slot, sub0, nsub)
    FINALS = [(s, c0, c1 - c0) for s in range(SLOTS) for (c0, c1) in SUPERS]

    with nc.Block() as block:

        @block.sync
        def _(sync):
            sync.dma_start(xhi[:, 0:2], d_xhi[:, 0:2]).then_inc(dsem, 16)
            sync.dma_start(xlo[:, 0:2], d_xlo[:, 0:2]).then_inc(dsem, 16)
            sync.dma_start(wf16[:], d_wf16[:]).then_inc(dsem, 16)
            sync.dma_start(wfp8[:], d_wfp8[:]).then_inc(dsem, 16)
            sync.dma_start(xhi[:, 2:4], d_xhi[:, 2:4]).then_inc(dsem, 16)
            sync.dma_start(xlo[:, 2:4], d_xlo[:, 2:4]).then_inc(dsem, 16)
            sync.dma_start(consts[:], d_consts[:]).then_inc(dsem, 16)
            sync.dma_start(x32[:, 0 : YCOLS // 2],
                           d_x32[:, 0 : YCOLS // 2]).then_inc(dsem, 16)
            sync.dma_start(x32[:, YCOLS // 2 : YCOLS],
                           d_x32[:, YCOLS // 2 : YCOLS]).then_inc(dsem, 16)
            # stats1 chain: AllReduce the [128,2] partials, fold halves after
            sync.wait_ge(s_st1, 1)
            sync.dma_start(db1_in[:], stats1[:, 0:2]).then_inc(dsem, 16)
            sync.wait_ge(s_cc, CCV)
            sync.dma_start(glob1[:, 0:2], db1_out[:]).then_inc(dsem, 16)
            sync.dma_start(glob1[0:64, 2:4], db1_out[64:128]).then_inc(dsem, 16)
            sync.dma_start(glob1[64:128, 2:4], db1_out[0:64]).then_inc(dsem, 16)
            # stats2 chain
            sync.wait_ge(s_st2, 1)
            sync.dma_start(db2_in[:], stats2[:, 0:2]).then_inc(dsem, 16)
            sync.wait_ge(s_cc, 2 * CCV)
            sync.dma_start(glob2[:, 0:2], db2_out[:]).then_inc(dsem, 16)
            sync.dma_start(glob2[0:64, 2:4], db2_out[64:128]).then_inc(dsem, 16)
            sync.dma_start(glob2[64:128, 2:4], db2_out[0:64]).then_inc(dsem, 16)
            # output stores (one per slot)
            for s in range(SLOTS):
                sync.wait_ge(s_fs, NSUP * (s + 1))
                sync.dma_start(
                    d_out[:, s * PERIMG : (s + 1) * PERIMG],
                    outv[:, OUTOFF + s * PERIMG : OUTOFF + (s + 1) * PERIMG]
                ).then_inc(dsem, 16)
            if DEBUG:
                sync.dma_start(d_g1[:], glob1[:]).then_inc(dsem, 16)
                sync.dma_start(d_g2[:], glob2[:]).then_inc(dsem, 16)
                sync.dma_start(d_y2[:], y2v[:, 0:YCOLS]).then_inc(dsem, 16)

        @block.tensor
        def _(tensor):
            # conv1: f16, 9 taps, two passes (hi + lo) into the same psum
            it = 0
            for q in range(QG):
                tensor.wait_ge(dsem, (D_QG0, D_QG1)[q])
                for (c0, c1) in SUPERS:
                    nsub = c1 - c0
                    if it >= 2:
                        tensor.wait_ge(s_eA1, it - 1)
                        tensor.wait_ge(s_eB1, it - 1)
                    pX = pbX[it % 2]
                    pY = pbY[it % 2]
                    quads = [
                        ((0, 0), slice(0, 64), 2 * q, pX, slice(0, 64)),
                        ((64, 0), slice(64, 128), 2 * q, pY, slice(0, 64)),
                        ((0, 64), slice(0, 64), 2 * q + 1, pX, slice(64, 128)),
                        ((64, 64), slice(64, 128), 2 * q + 1, pY,
                         slice(64, 128)),
                    ]
                    for tap in range(9):
                        kh, kw = tap // 3, tap % 3
                        wcol = tap * 64
                        for tp, rows, _, _, _ in quads:
                            nc.tensor.ldweights(wf16[rows, wcol : wcol + 64],
                                                tile_position=tp)
                        for ip, rhs_t in enumerate([xhi, xlo]):
                            for tp, rows, dslot, pdst, phalf in quads:
                                for s in range(nsub):
                                    c = c0 + s
                                    first = ip == 0 and tap == 0
                                    last = ip == 1 and tap == 8
                                    rap = rhs_t[rows, dslot,
                                                c * CHROWS + kh :
                                                c * CHROWS + kh + CHROWS,
                                                kw : kw + W]
                                    nc.tensor.matmul(
                                        pdst[phalf, s * 512 : s * 512 + CHUNK],
                                        wf16[rows, wcol : wcol + 64], rap,
                                        start=first, stop=last,
                                        tile_position=tp,
                                        skip_group_check=True)
                    tensor.drain().then_inc(s_pe1, 1)
                    it += 1

            # conv2: full-width fp8 DoubleRow (block-diagonal weights handle
            # both images of a slot per matmul), slot-major, 16 iterations
            PSUMS = [pbX[0], pbY[0], pbX[1], pbY[1]]
            wg0 = wfp8[:, 0:256].rearrange("p (a b) -> p a b", a=2)
            nc.tensor.ldweights(wg0, perf_mode=DR)
            for it in range(NIT2):
                slot, ci = it // NSUP, it % NSUP
                c0, c1 = SUPERS[ci]
                nsub = c1 - c0
                tensor.wait_ge(s_sg1, it + 1)
                if it >= 4:
                    if it % 2 == 0:
                        tensor.wait_ge(s_eA2, (it - 4) // 2 + 1)
                    else:
                        tensor.wait_ge(s_eB2, (it - 5) // 2 + 1)
                pdst = PSUMS[it % 4]
                v = bin1[:, slot]
                pstride = v.ap[0][0]
                for g, (khA, kwA, _) in enumerate(C2GROUPS):
                    wg = wfp8[:, g * 256 : (g + 1) * 256].rearrange(
                        "p (a b) -> p a b", a=2)
                    nc.tensor.ldweights(wg, perf_mode=DR)
                    for s in range(nsub):
                        c = c0 + s
                        off = v.offset + (c * CHROWS + khA) * BP + kwA
                        rap = AP(tensor=v.tensor, offset=off,
                                 ap=[[pstride, 128], [BP, 2], [1, BCHUNK]])
                        nc.tensor.matmul(
                            pdst[:, s * 512 : (s + 1) * 512],
                            wg, rap,
                            start=(g == 0), stop=(g == NG2 - 1),
                            perf_mode=DR,
                            skip_group_check=True)
                tensor.drain().then_inc(s_pe2, 1)

        @block.scalar
        def _(scalar):
            # conv1 evac: ACT reads pbX (slots 2q), sum accum into sa1
            it = 0
            for q in range(QG):
                for (c0, c1) in SUPERS:
                    nsub = c1 - c0
                    scalar.wait_ge(s_pe1, it + 1)
                    pX = pbX[it % 2]
                    src = pX[:, 0 : nsub * 512].rearrange(
                        "p (s k) -> p s k", s=nsub)[:, :, 0:CHUNK]
                    nc.scalar.activation(
                        y1[:, ycol(2 * q, c0) : ycol(2 * q, c0) + nsub * CHUNK],
                        src, AF.Copy,
                        accum_out=sa1[:, it : it + 1])
                    scalar.drain().then_inc(s_eA1, 1)
                    it += 1
            # stats1: sqrt(var + eps)
            scalar.wait_ge(s_st1, 2)
            nc.scalar.activation(glob1[:, 4:5], glob1[:, 5:6], AF.Sqrt)
            scalar.drain().then_inc(s_acst, 1)
            # conv2 evac: ACT reads the pbX-rotation iters (even its),
            # per-subchunk so the 8 junk cols per row are skipped
            PSUMS_S = [pbX[0], pbY[0], pbX[1], pbY[1]]
            eop = 0
            for it in range(NIT2):
                if it % 2 != 0:
                    continue
                slot, ci = it // NSUP, it % NSUP
                c0, c1 = SUPERS[ci]
                nsub = c1 - c0
                scalar.wait_ge(s_pe2, it + 1)
                pt = PSUMS_S[it % 4][:]
                pstride = pt.ap[0][0]
                for s in range(nsub):
                    src = AP(tensor=pt.tensor, offset=pt.offset + s * 512,
                             ap=[[pstride, 128], [BP, CHROWS], [1, W]])
                    # y2'' = y2' - S/2, so y2 = 2*y2'' exactly (S is even)
                    nc.scalar.activation(
                        y2v[:, ycol(slot, c0 + s) :
                            ycol(slot, c0 + s) + CHUNK],
                        src, AF.Identity, bias=consts[:, 4:5],
                        accum_out=sa2[:, eop : eop + 1])
                    eop += 1
                scalar.drain().then_inc(s_eA2, 1)
            # stats2 sqrt
            scalar.wait_ge(s_st2, 2)
            nc.scalar.activation(glob2[:, 4:5], glob2[:, 5:6], AF.Sqrt)
            scalar.drain().then_inc(s_acst, 2)
            # final: sign2 = Sign(w + bias2')
            for j in range(NF):
                sl, c0, nsub = FINALS[j]
                scalar.wait_ge(s_fv, j + 1)
                nc.scalar.activation(
                    outv[:, OUTOFF + ycol(sl, c0) :
                         OUTOFF + ycol(sl, c0) + nsub * CHUNK],
                    wbuf[j % 2][:, 0 : nsub * CHUNK], AF.Sign,
                    bias=glob2[:, 7:8])
                scalar.drain().then_inc(s_fs, 1)

        @block.vector
        def _(vector):
            # conv1: DVE evacs pbY (slots 2q+1) + sumsq over both slots
            it = 0
            for q in range(QG):
                for (c0, c1) in SUPERS:
                    nsub = c1 - c0
                    vector.wait_ge(s_pe1, it + 1)
                    pY = pbY[it % 2]
                    src = pY[:, 0 : nsub * 512].rearrange(
                        "p (s k) -> p s k", s=nsub)[:, :, 0:CHUNK]
                    nc.vector.tensor_scalar(
                        out=y1[:, ycol(2 * q + 1, c0) :
                               ycol(2 * q + 1, c0) + nsub * CHUNK],
                        in0=src, scalar1=0.0, scalar2=None,
                        op0=ALU.add, op1=ALU.add,
                        accum_out=sb1[:, it : it + 1])
                    nc.vector.drain().then_inc(s_eB1, 1)
                    vector.wait_ge(s_eA1, it + 1)
                    for half, slot in ((0, 2 * q), (1, 2 * q + 1)):
                        yc = y1[:, ycol(slot, c0) :
                                ycol(slot, c0) + nsub * CHUNK]
                        nc.vector.scalar_tensor_tensor(
                            out=scr[it % 2][:, 0 : nsub * CHUNK], in0=yc,
                            scalar=1.0, in1=yc,
                            op0=ALU.mult, op1=ALU.mult,
                            accum_out=qq1[:, 2 * it + half :
                                          2 * it + half + 1])
                    nc.vector.drain().then_inc(s_sq1, 1)
                    it += 1

            # stats1 fold + math: a1 = g1*rsqrt(var+eps), nb1 = m*a1 - b1
            nc.vector.reduce_sum(stats1[:, 6:7], sa1[:], axis=mybir.AxisListType.X)
            nc.vector.reduce_sum(stats1[:, 7:8], sb1[:], axis=mybir.AxisListType.X)
            nc.vector.reduce_sum(stats1[:, 1:2], qq1[:], axis=mybir.AxisListType.X)
            nc.vector.drain()
            nc.vector.tensor_tensor(out=stats1[:, 0:1], in0=stats1[:, 6:7],
                                    in1=stats1[:, 7:8], op=ALU.add)
            nc.vector.drain().then_inc(s_st1, 1)
            vector.wait_ge(dsem, D_G1)
            nc.vector.tensor_tensor(out=glob1[:, 0:2], in0=glob1[:, 0:2],
                                    in1=glob1[:, 2:4], op=ALU.add)
            nc.vector.drain()
            nc.vector.tensor_scalar_mul(glob1[:, 2:4], glob1[:, 0:2],
                                        1.0 / N_TOT)
            nc.vector.drain()
            nc.vector.tensor_tensor(out=glob1[:, 4:5], in0=glob1[:, 2:3],
                                    in1=glob1[:, 2:3], op=ALU.mult)
            nc.vector.drain()
            nc.vector.tensor_tensor(out=glob1[:, 5:6], in0=glob1[:, 3:4],
                                    in1=glob1[:, 4:5], op=ALU.subtract)
            nc.vector.drain()
            nc.vector.tensor_scalar_add(glob1[:, 5:6], glob1[:, 5:6], EPS)
            nc.vector.drain().then_inc(s_st1, 1)
            vector.wait_ge(s_acst, 1)
            nc.vector.reciprocal(glob1[:, 3:4], glob1[:, 4:5])
            nc.vector.drain()
            nc.vector.tensor_tensor(out=glob1[:, 6:7], in0=glob1[:, 3:4],
                                    in1=consts[:, 0:1], op=ALU.mult)
            nc.vector.drain()
            nc.vector.tensor_tensor(out=glob1[:, 4:5], in0=glob1[:, 2:3],
                                    in1=glob1[:, 6:7], op=ALU.mult)
            nc.vector.drain()
            nc.vector.tensor_tensor(out=glob1[:, 7:8], in0=glob1[:, 4:5],
                                    in1=consts[:, 1:2], op=ALU.subtract)
            nc.vector.drain().then_inc(s_m1, 1)

            # conv2: DVE evacs the pbY-rotation iters (odd its) + f16 sumsq
            # over every iter's fresh y2' columns (scaled by 1/64)
            PSUMS_V = [pbX[0], pbY[0], pbX[1], pbY[1]]
            eop = 0
            for it in range(NIT2):
                slot, ci = it // NSUP, it % NSUP
                c0, c1 = SUPERS[ci]
                nsub = c1 - c0
                if it % 2 == 1:
                    vector.wait_ge(s_pe2, it + 1)
                    pt = PSUMS_V[it % 4][:]
                    pstride = pt.ap[0][0]
                    for s in range(nsub):
                        src = AP(tensor=pt.tensor, offset=pt.offset + s * 512,
                                 ap=[[pstride, 128], [BP, CHROWS], [1, W]])
                        nc.vector.tensor_scalar(
                            out=y2v[:, ycol(slot, c0 + s) :
                                    ycol(slot, c0 + s) + CHUNK],
                            in0=src, scalar1=consts[:, 4:5], scalar2=None,
                            op0=ALU.add, op1=ALU.add,
                            accum_out=sb2[:, eop : eop + 1])
                        eop += 1
                    nc.vector.drain().then_inc(s_eB2, 1)
                else:
                    vector.wait_ge(s_eA2, it // 2 + 1)
                yc = y2v[:, ycol(slot, c0) : ycol(slot, c0) + nsub * CHUNK]
                nc.vector.scalar_tensor_tensor(
                    out=scr16[it % 2][:, 0 : nsub * CHUNK], in0=yc,
                    scalar=1.0 / 64.0, in1=yc,
                    op0=ALU.mult, op1=ALU.mult,
                    accum_out=qq2[:, it : it + 1])
                nc.vector.drain().then_inc(s_sq2, 1)

            # stats2 fold + math: y2 = 2*y2'' exactly, so
            #   m2 = 2*m'' ; var2 = 256*q'' - (2*m'')^2 ; SC = 2*g2*rsqrt(var2+eps)
            nc.vector.reduce_sum(stats2[:, 6:7], sa2[:], axis=mybir.AxisListType.X)
            nc.vector.reduce_sum(stats2[:, 7:8], sb2[:], axis=mybir.AxisListType.X)
            nc.vector.reduce_sum(stats2[:, 1:2], qq2[:], axis=mybir.AxisListType.X)
            nc.vector.drain()
            nc.vector.tensor_tensor(out=stats2[:, 0:1], in0=stats2[:, 6:7],
                                    in1=stats2[:, 7:8], op=ALU.add)
            nc.vector.drain().then_inc(s_st2, 1)
            vector.wait_ge(dsem, D_G2)
            nc.vector.tensor_tensor(out=glob2[:, 0:2], in0=glob2[:, 0:2],
                                    in1=glob2[:, 2:4], op=ALU.add)
            nc.vector.drain()
            nc.vector.tensor_scalar_mul(glob2[:, 2:4], glob2[:, 0:2],
                                        1.0 / N_TOT)
            nc.vector.drain()
            # col4 = (2*m'')^2 ; col5 = 256*q'' - col4 + eps = var2 + eps
            nc.vector.tensor_scalar_mul(glob2[:, 4:5], glob2[:, 2:3], 2.0)
            nc.vector.drain()
            nc.vector.tensor_tensor(out=glob2[:, 4:5], in0=glob2[:, 4:5],
                                    in1=glob2[:, 4:5], op=ALU.mult)
            nc.vector.drain()
            nc.vector.scalar_tensor_tensor(
                out=glob2[:, 5:6], in0=glob2[:, 3:4], scalar=256.0,
                in1=glob2[:, 4:5], op0=ALU.mult, op1=ALU.subtract)
            nc.vector.drain()
            nc.vector.tensor_scalar_add(glob2[:, 5:6], glob2[:, 5:6], EPS)
            nc.vector.drain().then_inc(s_st2, 1)
            vector.wait_ge(s_acst, 2)
            nc.vector.reciprocal(glob2[:, 3:4], glob2[:, 4:5])
            nc.vector.drain()
            # col6 = A2 = recip * g2 ; col7 = bias2' = beta2 - 2*A2*m'' ;
            # then col6 = SC = 2*A2  (m'' still lives in col2)
            nc.vector.tensor_tensor(out=glob2[:, 6:7], in0=glob2[:, 3:4],
                                    in1=consts[:, 2:3], op=ALU.mult)
            nc.vector.drain()
            nc.vector.tensor_tensor(out=glob2[:, 4:5], in0=glob2[:, 6:7],
                                    in1=glob2[:, 2:3], op=ALU.mult)
            nc.vector.drain()
            nc.vector.tensor_scalar_mul(glob2[:, 4:5], glob2[:, 4:5], 2.0)
            nc.vector.drain()
            nc.vector.tensor_tensor(out=glob2[:, 7:8], in0=consts[:, 3:4],
                                    in1=glob2[:, 4:5], op=ALU.subtract)
            nc.vector.tensor_scalar_mul(glob2[:, 6:7], glob2[:, 6:7], 2.0)
            nc.vector.drain()

            # final: w = SC*y2'' + x32 in f32 (single pass; bias2'
            # is applied inside the ACT Sign at f32 precision)
            for j in range(NF):
                sl, c0, nsub = FINALS[j]
                cols = slice(ycol(sl, c0), ycol(sl, c0) + nsub * CHUNK)
                if j >= 2:
                    vector.wait_ge(s_fs, j - 1)
                nc.vector.scalar_tensor_tensor(
                    out=wbuf[j % 2][:, 0 : nsub * CHUNK],
                    in0=y2v[:, cols],
                    scalar=glob2[:, 6:7],
                    in1=x32[:, cols],
                    op0=ALU.mult, op1=ALU.add)
                nc.vector.drain().then_inc(s_fv, 1)

        @block.gpsimd
        def _(gpsimd):
            # bin1 halo = 0.5 (== (0+1)/2, the zero-pad in {0,1} space)
            for s in range(SLOTS):
                nc.gpsimd.memset(bin1[:, s], 0.5)
            gpsimd.drain()
            gpsimd.wait_ge(dsem, D_B1DBIN)
            if CC_STUB:
                nc.gpsimd.dma_start(db1_out[:], db1_in[:]).then_inc(s_cc, 16)
            else:
                nc.gpsimd.collective_compute(
                    "AllReduce", ALU.add, replica_groups=[list(range(N_CORES))],
                    ins=[db1_in[:]], outs=[db1_out[:]]).then_inc(s_cc, 1)
            # sign1: bin1 = (a1*y1 >= -b1) in {0,1}, written as fp8.
            # Slot-major emission matches conv2's iteration order.
            gpsimd.wait_ge(s_m1, 1)
            for slot in range(SLOTS):
                for (c0, c1) in SUPERS:
                    nsub = c1 - c0
                    nc.gpsimd.tensor_scalar(
                        out=bin1[:, slot, 1 + c0 * CHROWS :
                                 1 + c1 * CHROWS, 1 : 1 + W],
                        in0=y1[:, ycol(slot, c0) :
                               ycol(slot, c0) + nsub * CHUNK],
                        scalar1=glob1[:, 6:7], scalar2=glob1[:, 7:8],
                        op0=ALU.mult, op1=ALU.is_ge)
                    gpsimd.drain().then_inc(s_sg1, 1)
            gpsimd.wait_ge(dsem, D_B2DBIN)
            if CC_STUB:
                nc.gpsimd.dma_start(db2_out[:], db2_in[:]).then_inc(s_cc, 16)
            else:
                nc.gpsimd.collective_compute(
                    "AllReduce", ALU.add, replica_groups=[list(range(N_CORES))],
                    ins=[db2_in[:]], outs=[db2_out[:]]).then_inc(s_cc, 1)

    return nc


_CACHE = {}


def _get_nc():
    if "nc" not in _CACHE:
        _CACHE["nc"] = build_bass()
    return _CACHE["nc"]


def kernel(x, w1, gamma1, beta1, w2, gamma2, beta2):
    x = np.asarray(x, np.float32)
    w1 = np.asarray(w1, np.float32)
    w2 = np.asarray(w2, np.float32)
    gamma1 = np.asarray(gamma1, np.float32)
    beta1 = np.asarray(beta1, np.float32)
    gamma2 = np.asarray(gamma2, np.float32)
    beta2 = np.asarray(beta2, np.float32)

    # conv1 weights: [tap, cin, cout] -> [cin, tap*cout], rows duplicated
    wb1 = np.where(w1 >= 0, 1.0, -1.0).astype(np.float32)
    wt1 = wb1.transpose(1, 2, 3, 0).reshape(64, 9, 64).reshape(64, 576)
    wf16_np = np.concatenate([wt1, wt1], axis=0).astype(np.float16)

    # conv2 weights: sign(w2) as fp8, full-width DoubleRow with
    # block-diagonal planes (two images per matmul, two kh-taps per pass)
    wb2 = np.where(w2 >= 0, 1.0, -1.0).astype(np.float32)   # [o, i, kh, kw]
    wtap = wb2.transpose(2, 3, 1, 0)                        # [kh, kw, i, o]
    wfp8_np = np.zeros((128, NG2 * 256), np.float32)
    for g, (khA, kwA, zeroA) in enumerate(C2GROUPS):
        for i in (0, 1):
            if i == 0 and zeroA:
                continue
            blk = np.zeros((128, 128), np.float32)
            blk[0:64, 0:64] = wtap[khA + i, kwA]
            blk[64:128, 64:128] = wtap[khA + i, kwA]
            wfp8_np[:, g * 256 + i * 128 : g * 256 + (i + 1) * 128] = blk
    wfp8_np = wfp8_np.astype(ml_dtypes.float8_e4m3)

    S = wb2.sum(axis=(1, 2, 3))                             # [64] per out-ch
    consts_np = np.zeros((128, 8), np.float32)
    for col, v in enumerate([gamma1, beta1, gamma2, beta2, -0.5 * S]):
        consts_np[0:64, col] = v
        consts_np[64:128, col] = v

    in_maps = []
    for k in range(N_CORES):
        xc = x[IMGS * k : IMGS * (k + 1)]            # [8, 64, 56, 56]
        xp = np.zeros((IMGS, C, HP, HP), np.float32)
        xp[:, :, 1 : 1 + H, 1 : 1 + W] = xc
        arr = xp.reshape(SLOTS, 2, C, HP, HP).transpose(1, 2, 0, 3, 4)
        arr = np.ascontiguousarray(arr).reshape(128, SLOTS, HP, HP)
        ahi = arr.astype(np.float16)
        alo = (arr - ahi.astype(np.float32)).astype(np.float16)
        # second x copy in y1's permuted slot order (interior only) for
        # the final residual add
        xperm = np.empty((2, C, SLOTS, H, W), np.float32)
        for s in range(SLOTS):
            for h in (0, 1):
                xperm[h, :, s] = xc[IMG_OF[s][h]]
        x32_np = np.ascontiguousarray(xperm.reshape(128, YCOLS))
        in_maps.append({
            "xhi": ahi, "xlo": alo, "x32": x32_np,
            "wf16": wf16_np, "wfp8": wfp8_np, "consts": consts_np,
        })

    nc = _get_nc()
    res = bass_utils.run_bass_kernel_spmd(nc, in_maps, core_ids=list(range(N_CORES)))

    out = np.empty((N, C, H, W), np.float32)
    for k in range(N_CORES):
        o = np.asarray(res.results[k]["outp"]).astype(np.float32)  # [128, 12544]
        o = o.reshape(2, C, SLOTS, H, W)
        for s in range(SLOTS):
            for h in (0, 1):
                out[IMGS * k + IMG_OF[s][h]] = o[h, :, s]
    return out


if __name__ == "__main__":
    rng = np.random.default_rng(0)
    xs = rng.standard_normal((N, C, H, W)).astype(np.float32)
    w1s = (rng.standard_normal((C, C, 3, 3)) * 0.1).astype(np.float32)
    w2s = (rng.standard_normal((C, C, 3, 3)) * 0.1).astype(np.float32)
    ones = np.ones(C, np.float32)
    zeros = np.zeros(C, np.float32)
    r = kernel(x=xs, w1=w1s, gamma1=ones, beta1=zeros, w2=w2s, gamma2=ones,
               beta2=zeros)
    print("ran, out uniq:", np.unique(r))


# revision 45
# speedup vs baseline: 1.3277x; 1.0152x over previous
"""BinaryBasicBlock TRN2 kernel: 8-core batch-parallel, raw Bass.

Reference computation (per core: 8 images, C=64, 56x56):
  y1   = conv3x3(x, sign(w1))            # exact: x = fp16(x) + fp16(residual)
  bin1 = sign((y1 - mu1) * rsqrt(var1+eps) * g1 + b1)   # global batch stats
  y2   = conv3x3(bin1, sign(w2))         # exact
  out  = sign((y2 - mu2) * rsqrt(var2+eps) * g2 + b2 + x)

Batch stats are exact: per-core (sum, sumsq) partials are AllReduced across
the 8 cores mid-kernel.

v2 speedups over the baseline:
  - conv2 runs in fp8e4 with perf_mode=DoubleRow: bin1 is stored as
    {0,1} (0.5 at the padding halo) so +-1 inputs become exact fp8; the
    0/1 offset is folded into per-channel scalars via S_o = sum(sign(w2))
    (y2 = 2*y2' - S).  Taps pair along kh (pair step 64B, %16-aligned).
  - bin1 row pitch is 64 so a conv2 matmul streams one contiguous
    512-element window (8 rows x 64); the 8 junk columns per row are
    skipped at PSUM evacuation.
  - sign1 (bin1 = is_ge(a1*y1, -b1)) runs on the otherwise-idle GPSIMD
    engine, freeing ACT/DVE in the conv2 phase.
  - PSUM evacuation is split: ACT always reads the pbX banks, DVE always
    reads the pbY banks (one PSUM reader engine per bank).
  - sumsq for conv2 stats and both final residual passes run as all-f16
    tensor_scalar_ptr ops on DVE (4x DVE perf mode).

Toolchain constraints honored: raw Bass only, max one semaphore wait per
instruction, single PSUM reader engine per bank, drain-backed semaphore
increments on every cross-engine RAW edge, explicit DVE drains between
dependent vector ops.
"""
import numpy as np
import ml_dtypes
import concourse.bass as bass
import concourse.mybir as mybir
from concourse.ap import AP
from concourse import bass_utils
from contextlib import ExitStack

F32 = mybir.dt.float32
BF16 = mybir.dt.bfloat16
F16 = mybir.dt.float16
FP8 = mybir.dt.float8e4
AF = mybir.ActivationFunctionType
ALU = mybir.AluOpType
DR = mybir.MatmulPerfMode.DoubleRow

N_CORES = 8
N, C, H, W = 64, 64, 56, 56
IMGS = N // N_CORES          # 8 images per core
SLOTS = IMGS // 2            # 4 slots (2 images per slot)
QG = SLOTS // 2              # 2 quadgroups (4 images each)
HP = H + 2                   # 58 padded
BROWS = H + 3                # 59 rows in the fp8 bin1 (1 extra guard row)
BP = 64                      # bin1 row pitch
CHROWS = 8                   # output rows per 448-subchunk
CHUNK = CHROWS * W           # 448
BCHUNK = CHROWS * BP         # 512 (conv2 psum cols per subchunk)
NCH = H // CHROWS            # 7 subchunks per image
SUPERS = [(0, 2), (2, 4), (4, 6), (6, 7)]   # subchunk ranges per super-iter
NSUP = len(SUPERS)           # 4 super-iters per quadgroup
ITERS = QG * NSUP            # 8 super-iters per conv
PERIMG = H * W               # 3136
YCOLS = SLOTS * PERIMG       # 12544
N_TOT = float(N * H * W)     # global batch-stat count
EPS = 1e-5
NF = SLOTS * NSUP            # 16 final-stage iterations (per-slot supers)

# conv2 DoubleRow tap groups: plane A at (khA, kwA), plane B at (khA+1, kwA).
# zeroA marks groups whose A-plane weights are zero (kh=2 taps ride alone).
C2GROUPS = [
    (0, 0, False),   # taps (0,0)+(1,0)
    (0, 1, False),   # taps (0,1)+(1,1)
    (0, 2, False),   # taps (0,2)+(1,2)
    (1, 0, True),    # zero + tap (2,0)
    (1, 1, True),    # zero + tap (2,1)
    (1, 2, True),    # zero + tap (2,2)
]
NG2 = len(C2GROUPS)
NIT2 = SLOTS * NSUP          # 16 conv2 iterations (slot-major, full-width)

# y1/bin1/y2 slot layout after conv1's quad permutation: slot 2q holds
# images (4q, 4q+2) on its partition halves, slot 2q+1 holds (4q+1, 4q+3).
IMG_OF = {}
for _q in range(QG):
    IMG_OF[2 * _q] = (4 * _q, 4 * _q + 2)
    IMG_OF[2 * _q + 1] = (4 * _q + 1, 4 * _q + 3)

DEBUG = False
CC_STUB = False   # replace AllReduce with a local DMA (for TimelineSim)


def build_bass():
    nc = bass.Bass(trn_type="TRN2", target_bir_lowering=False, debug=False,
                   num_devices=N_CORES)

    d_xhi = nc.dram_tensor("xhi", [128, SLOTS, HP, HP], F16, kind="ExternalInput")
    d_xlo = nc.dram_tensor("xlo", [128, SLOTS, HP, HP], F16, kind="ExternalInput")
    d_x32 = nc.dram_tensor("x32", [128, YCOLS], F32, kind="ExternalInput")
    d_wf16 = nc.dram_tensor("wf16", [128, 576], F16, kind="ExternalInput")
    d_wfp8 = nc.dram_tensor("wfp8", [128, NG2 * 256], FP8, kind="ExternalInput")
    d_consts = nc.dram_tensor("consts", [128, 8], F32, kind="ExternalInput")
    d_out = nc.dram_tensor("outp", [128, YCOLS], BF16, kind="ExternalOutput")
    db1_in = nc.dram_tensor("db1_in", [128, 2], F32)
    db1_out = nc.dram_tensor("db1_out", [128, 2], F32, addr_space="Shared")
    db2_in = nc.dram_tensor("db2_in", [128, 2], F32)
    db2_out = nc.dram_tensor("db2_out", [128, 2], F32, addr_space="Shared")
    if DEBUG:
        d_g1 = nc.dram_tensor("dbg_g1", [128, 8], F32, kind="ExternalOutput")
        d_g2 = nc.dram_tensor("dbg_g2", [128, 8], F32, kind="ExternalOutput")
        d_y2 = nc.dram_tensor("dbg_y2", [128, YCOLS], F16, kind="ExternalOutput")

    es = ExitStack()
    def sb(name, shape, dt):
        return es.enter_context(nc.sbuf_tensor(name, shape, dt))
    def ps(name, shape, dt):
        return es.enter_context(nc.psum_tensor(name, shape, dt))
    def sem(name):
        return es.enter_context(nc.semaphore(name))

    xhi = sb("xhi_t", [128, SLOTS, HP, HP], F16)
    xlo = sb("xlo_t", [128, SLOTS, HP, HP], F16)
    x32 = sb("x32_t", [128, YCOLS], F32)
    wf16 = sb("wf16_t", [128, 576], F16)
    wfp8 = sb("wfp8_t", [128, NG2 * 256], FP8)
    consts = sb("consts_t", [128, 8], F32)
    bin1 = sb("bin1_t", [128, SLOTS, BROWS, BP], FP8)
    y1 = sb("y1_t", [128, YCOLS], F32)
    # y2 (fp16) and the output (bf16) live in y1's bytes (dead by then)
    y2v = y1[:].bitcast(F16)      # [128, 25088] f16 ; cols 0..12543 used
    outv = y1[:].bitcast(BF16)    # [128, 25088] bf16; cols 12544..25087 used
    OUTOFF = YCOLS
    sa1 = sb("sa1", [128, ITERS], F32)
    sb1 = sb("sb1", [128, ITERS], F32)
    qq1 = sb("qq1", [128, 2 * ITERS], F32)
    # conv2 evac op counts: ACT handles even iters, DVE odd iters
    EOPA = sum(SUPERS[it % NSUP][1] - SUPERS[it % NSUP][0]
               for it in range(NIT2) if it % 2 == 0)
    EOPB = sum(SUPERS[it % NSUP][1] - SUPERS[it % NSUP][0]
               for it in range(NIT2) if it % 2 == 1)
    sa2 = sb("sa2", [128, EOPA], F32)
    sb2 = sb("sb2", [128, EOPB], F32)
    qq2 = sb("qq2", [128, NIT2], F32)
    stats1 = sb("stats1", [128, 8], F32)
    stats2 = sb("stats2", [128, 8], F32)
    glob1 = sb("glob1", [128, 8], F32)
    glob2 = sb("glob2", [128, 8], F32)
    scr = [sb(f"scr{i}", [128, 2 * CHUNK], F32) for i in range(2)]
    scr16 = [s[:].bitcast(F16) for s in scr]
    wbuf = [sb(f"wb{i}", [128, 4 * CHUNK], F32) for i in range(2)]
    scrA = sb("scrA", [128, 2 * CHUNK], F32)
    # PSUM: 2 sets x (X, Y) tensors of 2 banks each = 8 banks
    pbX = [ps(f"pbX{i}", [128, 1024], F32) for i in range(2)]
    pbY = [ps(f"pbY{i}", [128, 1024], F32) for i in range(2)]

    dsem = sem("dsem")
    s_pe1 = sem("s_pe1"); s_pe2 = sem("s_pe2")
    s_eA1 = sem("s_eA1"); s_eB1 = sem("s_eB1")
    s_eA2 = sem("s_eA2"); s_eB2 = sem("s_eB2")
    s_sq1 = sem("s_sq1"); s_sq2 = sem("s_sq2")
    s_st1 = sem("s_st1"); s_st2 = sem("s_st2"); s_acst = sem("s_acst")
    s_m1 = sem("s_m1")
    s_sg1 = sem("s_sg1"); s_sgA = sem("s_sgA"); s_ms = sem("s_ms")
    s_qa = sem("s_qa")
    s_cc = sem("s_cc")
    s_fv = sem("s_fv"); s_fs = sem("s_fs")

    CCV = 16 if CC_STUB else 1
    # dsem milestones (each DMA increments by 16)
    D_QG0 = 4 * 16      # xhi01, xlo01, wf16, wfp8
    D_QG1 = 7 * 16      # xhi23, xlo23, consts
    D_XP = 9 * 16       # x32 halves
    D_B1DBIN = 10 * 16
    D_G1 = 13 * 16      # allreduce-1 result + swapped halves loaded
    D_B2DBIN = 14 * 16
    D_G2 = 17 * 16

    def ycol(slot, c):
        return slot * PERIMG + c * CHUNK

    # final-stage iteration table: (slot, sub0, nsub) — 8 bigger chunks
    FINALS = [(s, c0, c1 - c0) for s in range(SLOTS)
              for (c0, c1) in ((0, 4), (4, 7))]
    NFIN = len(FINALS)

    with nc.Block() as block:

        @block.sync
        def _(sync):
            sync.dma_start(xhi[:, 0:2], d_xhi[:, 0:2]).then_inc(dsem, 16)
            sync.dma_start(xlo[:, 0:2], d_xlo[:, 0:2]).then_inc(dsem, 16)
            sync.dma_start(wf16[:], d_wf16[:]).then_inc(dsem, 16)
            sync.dma_start(wfp8[:], d_wfp8[:]).then_inc(dsem, 16)
            sync.dma_start(xhi[:, 2:4], d_xhi[:, 2:4]).then_inc(dsem, 16)
            sync.dma_start(xlo[:, 2:4], d_xlo[:, 2:4]).then_inc(dsem, 16)
            sync.dma_start(consts[:], d_consts[:]).then_inc(dsem, 16)
            sync.dma_start(x32[:, 0 : YCOLS // 2],
                           d_x32[:, 0 : YCOLS // 2]).then_inc(dsem, 16)
            sync.dma_start(x32[:, YCOLS // 2 : YCOLS],
                           d_x32[:, YCOLS // 2 : YCOLS]).then_inc(dsem, 16)
            # stats1 chain: AllReduce the [128,2] partials, fold halves after
            sync.wait_ge(s_st1, 1)
            sync.dma_start(db1_in[:], stats1[:, 0:2]).then_inc(dsem, 16)
            sync.wait_ge(s_cc, CCV)
            sync.dma_start(glob1[:, 0:2], db1_out[:]).then_inc(dsem, 16)
            sync.dma_start(glob1[0:64, 2:4], db1_out[64:128]).then_inc(dsem, 16)
            sync.dma_start(glob1[64:128, 2:4], db1_out[0:64]).then_inc(dsem, 16)
            # stats2 chain
            sync.wait_ge(s_st2, 1)
            sync.dma_start(db2_in[:], stats2[:, 0:2]).then_inc(dsem, 16)
            sync.wait_ge(s_cc, 2 * CCV)
            sync.dma_start(glob2[:, 0:2], db2_out[:]).then_inc(dsem, 16)
            sync.dma_start(glob2[0:64, 2:4], db2_out[64:128]).then_inc(dsem, 16)
            sync.dma_start(glob2[64:128, 2:4], db2_out[0:64]).then_inc(dsem, 16)
            # output stores (one per slot)
            for s in range(SLOTS):
                sync.wait_ge(s_fs, 2 * (s + 1))
                sync.dma_start(
                    d_out[:, s * PERIMG : (s + 1) * PERIMG],
                    outv[:, OUTOFF + s * PERIMG : OUTOFF + (s + 1) * PERIMG]
                ).then_inc(dsem, 16)
            if DEBUG:
                sync.dma_start(d_g1[:], glob1[:]).then_inc(dsem, 16)
                sync.dma_start(d_g2[:], glob2[:]).then_inc(dsem, 16)
                sync.dma_start(d_y2[:], y2v[:, 0:YCOLS]).then_inc(dsem, 16)

        @block.tensor
        def _(tensor):
            # conv1: f16, 9 taps, two passes (hi + lo) into the same psum
            it = 0
            for q in range(QG):
                tensor.wait_ge(dsem, (D_QG0, D_QG1)[q])
                for (c0, c1) in SUPERS:
                    nsub = c1 - c0
                    if it >= 2:
                        tensor.wait_ge(s_eA1, it - 1)
                        tensor.wait_ge(s_eB1, it - 1)
                    pX = pbX[it % 2]
                    pY = pbY[it % 2]
                    quads = [
                        ((0, 0), slice(0, 64), 2 * q, pX, slice(0, 64)),
                        ((64, 0), slice(64, 128), 2 * q, pY, slice(0, 64)),
                        ((0, 64), slice(0, 64), 2 * q + 1, pX, slice(64, 128)),
                        ((64, 64), slice(64, 128), 2 * q + 1, pY,
                         slice(64, 128)),
                    ]
                    for tap in range(9):
                        kh, kw = tap // 3, tap % 3
                        wcol = tap * 64
                        for tp, rows, _, _, _ in quads:
                            nc.tensor.ldweights(wf16[rows, wcol : wcol + 64],
                                                tile_position=tp)
                        for ip, rhs_t in enumerate([xhi, xlo]):
                            for tp, rows, dslot, pdst, phalf in quads:
                                for s in range(nsub):
                                    c = c0 + s
                                    first = ip == 0 and tap == 0
                                    last = ip == 1 and tap == 8
                                    rap = rhs_t[rows, dslot,
                                                c * CHROWS + kh :
                                                c * CHROWS + kh + CHROWS,
                                                kw : kw + W]
                                    nc.tensor.matmul(
                                        pdst[phalf, s * 512 : s * 512 + CHUNK],
                                        wf16[rows, wcol : wcol + 64], rap,
                                        start=first, stop=last,
                                        tile_position=tp,
                                        skip_group_check=True)
                    tensor.drain().then_inc(s_pe1, 1)
                    it += 1

            # conv2: full-width fp8 DoubleRow (block-diagonal weights handle
            # both images of a slot per matmul), slot-major, 16 iterations
            PSUMS = [pbX[0], pbY[0], pbX[1], pbY[1]]
            wg0 = wfp8[:, 0:256].rearrange("p (a b) -> p a b", a=2)
            nc.tensor.ldweights(wg0, perf_mode=DR)
            for it in range(NIT2):
                slot, ci = it // NSUP, it % NSUP
                c0, c1 = SUPERS[ci]
                nsub = c1 - c0
                tensor.wait_ge(s_sg1, it + 1)
                if it >= 4:
                    if it % 2 == 0:
                        tensor.wait_ge(s_eA2, (it - 4) // 2 + 1)
                    else:
                        tensor.wait_ge(s_eB2, (it - 5) // 2 + 1)
                pdst = PSUMS[it % 4]
                v = bin1[:, slot]
                pstride = v.ap[0][0]
                for g, (khA, kwA, _) in enumerate(C2GROUPS):
                    wg = wfp8[:, g * 256 : (g + 1) * 256].rearrange(
                        "p (a b) -> p a b", a=2)
                    nc.tensor.ldweights(wg, perf_mode=DR)
                    for s in range(nsub):
                        c = c0 + s
                        off = v.offset + (c * CHROWS + khA) * BP + kwA
                        rap = AP(tensor=v.tensor, offset=off,
                                 ap=[[pstride, 128], [BP, 2], [1, BCHUNK]])
                        nc.tensor.matmul(
                            pdst[:, s * 512 : (s + 1) * 512],
                            wg, rap,
                            start=(g == 0), stop=(g == NG2 - 1),
                            perf_mode=DR,
                            skip_group_check=True)
                tensor.drain().then_inc(s_pe2, 1)

        @block.scalar
        def _(scalar):
            # conv1 evac: ACT reads pbX (slots 2q), sum accum into sa1
            it = 0
            for q in range(QG):
                for (c0, c1) in SUPERS:
                    nsub = c1 - c0
                    scalar.wait_ge(s_pe1, it + 1)
                    pX = pbX[it % 2]
                    src = pX[:, 0 : nsub * 512].rearrange(
                        "p (s k) -> p s k", s=nsub)[:, :, 0:CHUNK]
                    nc.scalar.activation(
                        y1[:, ycol(2 * q, c0) : ycol(2 * q, c0) + nsub * CHUNK],
                        src, AF.Copy,
                        accum_out=sa1[:, it : it + 1])
                    scalar.drain().then_inc(s_eA1, 1)
                    it += 1
            # ACT picks up the last super's sumsq so DVE's tail is short
            scalar.wait_ge(s_eB1, 8)
            for half, slot in ((0, 2), (1, 3)):
                yc = y1[:, ycol(slot, 6) : ycol(slot, 6) + CHUNK]
                nc.scalar.activation(
                    scrA[:, 0:CHUNK], yc, AF.Square,
                    accum_out=qq1[:, 14 + half : 15 + half])
            scalar.drain().then_inc(s_qa, 1)
            # stats1: sqrt(var + eps)
            scalar.wait_ge(s_st1, 2)
            nc.scalar.activation(glob1[:, 4:5], glob1[:, 5:6], AF.Sqrt)
            scalar.drain().then_inc(s_acst, 1)
            # conv2 evac: ACT reads the pbX-rotation iters (even its),
            # per-subchunk so the 8 junk cols per row are skipped
            PSUMS_S = [pbX[0], pbY[0], pbX[1], pbY[1]]
            eop = 0
            for it in range(NIT2):
                if it % 2 != 0:
                    continue
                slot, ci = it // NSUP, it % NSUP
                c0, c1 = SUPERS[ci]
                nsub = c1 - c0
                scalar.wait_ge(s_pe2, it + 1)
                pt = PSUMS_S[it % 4][:]
                pstride = pt.ap[0][0]
                for s in range(nsub):
                    src = AP(tensor=pt.tensor, offset=pt.offset + s * 512,
                             ap=[[pstride, 128], [BP, CHROWS], [1, W]])
                    # y2'' = y2' - S/2, so y2 = 2*y2'' exactly (S is even)
                    nc.scalar.activation(
                        y2v[:, ycol(slot, c0 + s) :
                            ycol(slot, c0 + s) + CHUNK],
                        src, AF.Identity, bias=consts[:, 4:5],
                        accum_out=sa2[:, eop : eop + 1])
                    eop += 1
                scalar.drain().then_inc(s_eA2, 1)
            # ACT squares for the last four iters' sumsq (scale 1/8 keeps
            # the 1/64-scaled accumulation convention)
            for it in (12, 13, 14, 15):
                slot, ci = it // NSUP, it % NSUP
                c0, c1 = SUPERS[ci]
                nsub = c1 - c0
                if it % 2 == 1:
                    scalar.wait_ge(s_eB2, (it + 1) // 2)
                yc = y2v[:, ycol(slot, c0) : ycol(slot, c0) + nsub * CHUNK]
                nc.scalar.activation(
                    scrA[:, 0 : nsub * CHUNK], yc, AF.Square, scale=0.125,
                    accum_out=qq2[:, it : it + 1])
            scalar.drain().then_inc(s_qa, 2)
            # stats2 sqrt
            scalar.wait_ge(s_st2, 2)
            nc.scalar.activation(glob2[:, 4:5], glob2[:, 5:6], AF.Sqrt)
            scalar.drain().then_inc(s_acst, 2)
            # final: sign2 = Sign(w + bias2')
            for j in range(NFIN):
                sl, c0, nsub = FINALS[j]
                scalar.wait_ge(s_fv, j + 1)
                nc.scalar.activation(
                    outv[:, OUTOFF + ycol(sl, c0) :
                         OUTOFF + ycol(sl, c0) + nsub * CHUNK],
                    wbuf[j % 2][:, 0 : nsub * CHUNK], AF.Sign,
                    bias=glob2[:, 7:8])
                scalar.drain().then_inc(s_fs, 1)

        @block.vector
        def _(vector):
            # conv1: DVE evacs pbY (slots 2q+1) + sumsq over both slots
            it = 0
            for q in range(QG):
                for (c0, c1) in SUPERS:
                    nsub = c1 - c0
                    vector.wait_ge(s_pe1, it + 1)
                    pY = pbY[it % 2]
                    src = pY[:, 0 : nsub * 512].rearrange(
                        "p (s k) -> p s k", s=nsub)[:, :, 0:CHUNK]
                    nc.vector.tensor_scalar(
                        out=y1[:, ycol(2 * q + 1, c0) :
                               ycol(2 * q + 1, c0) + nsub * CHUNK],
                        in0=src, scalar1=0.0, scalar2=None,
                        op0=ALU.add, op1=ALU.add,
                        accum_out=sb1[:, it : it + 1])
                    nc.vector.drain().then_inc(s_eB1, 1)
                    if it < 7:
                        vector.wait_ge(s_eA1, it + 1)
                        for half, slot in ((0, 2 * q), (1, 2 * q + 1)):
                            yc = y1[:, ycol(slot, c0) :
                                    ycol(slot, c0) + nsub * CHUNK]
                            nc.vector.scalar_tensor_tensor(
                                out=scr[it % 2][:, 0 : nsub * CHUNK], in0=yc,
                                scalar=1.0, in1=yc,
                                op0=ALU.mult, op1=ALU.mult,
                                accum_out=qq1[:, 2 * it + half :
                                              2 * it + half + 1])
                        nc.vector.drain()
                    it += 1

            # stats1 fold + math: a1 = g1*rsqrt(var+eps), nb1 = m*a1 - b1
            vector.wait_ge(s_qa, 1)
            nc.vector.reduce_sum(stats1[:, 6:7], sa1[:], axis=mybir.AxisListType.X)
            nc.vector.reduce_sum(stats1[:, 7:8], sb1[:], axis=mybir.AxisListType.X)
            nc.vector.reduce_sum(stats1[:, 1:2], qq1[:], axis=mybir.AxisListType.X)
            nc.vector.drain()
            nc.vector.tensor_tensor(out=stats1[:, 0:1], in0=stats1[:, 6:7],
                                    in1=stats1[:, 7:8], op=ALU.add)
            nc.vector.drain().then_inc(s_st1, 1)
            vector.wait_ge(dsem, D_G1)
            nc.vector.tensor_tensor(out=glob1[:, 0:2], in0=glob1[:, 0:2],
                                    in1=glob1[:, 2:4], op=ALU.add)
            nc.vector.drain()
            nc.vector.tensor_scalar_mul(glob1[:, 2:4], glob1[:, 0:2],
                                        1.0 / N_TOT)
            nc.vector.drain()
            nc.vector.tensor_tensor(out=glob1[:, 4:5], in0=glob1[:, 2:3],
                                    in1=glob1[:, 2:3], op=ALU.mult)
            nc.vector.drain()
            nc.vector.tensor_tensor(out=glob1[:, 5:6], in0=glob1[:, 3:4],
                                    in1=glob1[:, 4:5], op=ALU.subtract)
            nc.vector.drain()
            nc.vector.tensor_scalar_add(glob1[:, 5:6], glob1[:, 5:6], EPS)
            nc.vector.drain().then_inc(s_st1, 1)
            vector.wait_ge(s_acst, 1)
            nc.vector.reciprocal(glob1[:, 3:4], glob1[:, 4:5])
            nc.vector.drain()
            nc.vector.tensor_tensor(out=glob1[:, 6:7], in0=glob1[:, 3:4],
                                    in1=consts[:, 0:1], op=ALU.mult)
            nc.vector.drain()
            nc.vector.tensor_tensor(out=glob1[:, 4:5], in0=glob1[:, 2:3],
                                    in1=glob1[:, 6:7], op=ALU.mult)
            nc.vector.drain()
            nc.vector.tensor_tensor(out=glob1[:, 7:8], in0=glob1[:, 4:5],
                                    in1=consts[:, 1:2], op=ALU.subtract)
            nc.vector.drain().then_inc(s_m1, 1)

            # conv2: DVE evacs the pbY-rotation iters (odd its) + f16 sumsq
            # over every iter's fresh y2' columns (scaled by 1/64)
            PSUMS_V = [pbX[0], pbY[0], pbX[1], pbY[1]]
            eop = 0
            for it in range(NIT2):
                slot, ci = it // NSUP, it % NSUP
                c0, c1 = SUPERS[ci]
                nsub = c1 - c0
                if it % 2 == 1:
                    vector.wait_ge(s_pe2, it + 1)
                    pt = PSUMS_V[it % 4][:]
                    pstride = pt.ap[0][0]
                    for s in range(nsub):
                        src = AP(tensor=pt.tensor, offset=pt.offset + s * 512,
                                 ap=[[pstride, 128], [BP, CHROWS], [1, W]])
                        nc.vector.tensor_scalar(
                            out=y2v[:, ycol(slot, c0 + s) :
                                    ycol(slot, c0 + s) + CHUNK],
                            in0=src, scalar1=consts[:, 4:5], scalar2=None,
                            op0=ALU.add, op1=ALU.add,
                            accum_out=sb2[:, eop : eop + 1])
                        eop += 1
                    nc.vector.drain().then_inc(s_eB2, 1)
                elif it < 12:
                    vector.wait_ge(s_eA2, it // 2 + 1)
                if it < 12:
                    yc = y2v[:, ycol(slot, c0) : ycol(slot, c0) + nsub * CHUNK]
                    nc.vector.scalar_tensor_tensor(
                        out=scr16[it % 2][:, 0 : nsub * CHUNK], in0=yc,
                        scalar=1.0 / 64.0, in1=yc,
                        op0=ALU.mult, op1=ALU.mult,
                        accum_out=qq2[:, it : it + 1])
                    nc.vector.drain()

            # stats2 fold + math: y2 = 2*y2'' exactly, so
            #   m2 = 2*m'' ; var2 = 256*q'' - (2*m'')^2 ; SC = 2*g2*rsqrt(var2+eps)
            vector.wait_ge(s_qa, 2)
            nc.vector.reduce_sum(stats2[:, 6:7], sa2[:], axis=mybir.AxisListType.X)
            nc.vector.reduce_sum(stats2[:, 7:8], sb2[:], axis=mybir.AxisListType.X)
            nc.vector.reduce_sum(stats2[:, 1:2], qq2[:], axis=mybir.AxisListType.X)
            nc.vector.drain()
            nc.vector.tensor_tensor(out=stats2[:, 0:1], in0=stats2[:, 6:7],
                                    in1=stats2[:, 7:8], op=ALU.add)
            nc.vector.drain().then_inc(s_st2, 1)
            vector.wait_ge(dsem, D_G2)
            nc.vector.tensor_tensor(out=glob2[:, 0:2], in0=glob2[:, 0:2],
                                    in1=glob2[:, 2:4], op=ALU.add)
            nc.vector.drain()
            nc.vector.tensor_scalar_mul(glob2[:, 2:4], glob2[:, 0:2],
                                        1.0 / N_TOT)
            nc.vector.drain()
            # col4 = (2*m'')^2 ; col5 = 256*q'' - col4 + eps = var2 + eps
            nc.vector.tensor_scalar_mul(glob2[:, 4:5], glob2[:, 2:3], 2.0)
            nc.vector.drain()
            nc.vector.tensor_tensor(out=glob2[:, 4:5], in0=glob2[:, 4:5],
                                    in1=glob2[:, 4:5], op=ALU.mult)
            nc.vector.drain()
            nc.vector.scalar_tensor_tensor(
                out=glob2[:, 5:6], in0=glob2[:, 3:4], scalar=256.0,
                in1=glob2[:, 4:5], op0=ALU.mult, op1=ALU.subtract)
            nc.vector.drain()
            nc.vector.tensor_scalar_add(glob2[:, 5:6], glob2[:, 5:6], EPS)
            nc.vector.drain().then_inc(s_st2, 1)
            vector.wait_ge(s_acst, 2)
            nc.vector.reciprocal(glob2[:, 3:4], glob2[:, 4:5])
            nc.vector.drain()
            # col6 = A2 = recip * g2 ; col7 = bias2' = beta2 - 2*A2*m'' ;
            # then col6 = SC = 2*A2  (m'' still lives in col2)
            nc.vector.tensor_tensor(out=glob2[:, 6:7], in0=glob2[:, 3:4],
                                    in1=consts[:, 2:3], op=ALU.mult)
            nc.vector.drain()
            nc.vector.tensor_tensor(out=glob2[:, 4:5], in0=glob2[:, 6:7],
                                    in1=glob2[:, 2:3], op=ALU.mult)
            nc.vector.drain()
            nc.vector.tensor_scalar_mul(glob2[:, 4:5], glob2[:, 4:5], 2.0)
            nc.vector.drain()
            nc.vector.tensor_tensor(out=glob2[:, 7:8], in0=consts[:, 3:4],
                                    in1=glob2[:, 4:5], op=ALU.subtract)
            nc.vector.tensor_scalar_mul(glob2[:, 6:7], glob2[:, 6:7], 2.0)
            nc.vector.drain()

            # final: w = SC*y2'' + x32 in f32 (single pass; bias2'
            # is applied inside the ACT Sign at f32 precision)
            for j in range(NFIN):
                sl, c0, nsub = FINALS[j]
                cols = slice(ycol(sl, c0), ycol(sl, c0) + nsub * CHUNK)
                if j >= 2:
                    vector.wait_ge(s_fs, j - 1)
                nc.vector.scalar_tensor_tensor(
                    out=wbuf[j % 2][:, 0 : nsub * CHUNK],
                    in0=y2v[:, cols],
                    scalar=glob2[:, 6:7],
                    in1=x32[:, cols],
                    op0=ALU.mult, op1=ALU.add)
                nc.vector.drain().then_inc(s_fv, 1)

        @block.gpsimd
        def _(gpsimd):
            # bin1 halo = 0.5 (== (0+1)/2, the zero-pad in {0,1} space)
            for s in range(SLOTS):
                nc.gpsimd.memset(bin1[:, s], 0.5)
            gpsimd.drain()
            gpsimd.wait_ge(dsem, D_B1DBIN)
            if CC_STUB:
                nc.gpsimd.dma_start(db1_out[:], db1_in[:]).then_inc(s_cc, 16)
            else:
                nc.gpsimd.collective_compute(
                    "AllReduce", ALU.add, replica_groups=[list(range(N_CORES))],
                    ins=[db1_in[:]], outs=[db1_out[:]]).then_inc(s_cc, 1)
            # sign1: bin1 = (a1*y1 >= -b1) in {0,1}, written as fp8.
            # Slot-major emission matches conv2's iteration order.
            gpsimd.wait_ge(s_m1, 1)
            for slot in range(SLOTS):
                for (c0, c1) in SUPERS:
                    nsub = c1 - c0
                    nc.gpsimd.tensor_scalar(
                        out=bin1[:, slot, 1 + c0 * CHROWS :
                                 1 + c1 * CHROWS, 1 : 1 + W],
                        in0=y1[:, ycol(slot, c0) :
                               ycol(slot, c0) + nsub * CHUNK],
                        scalar1=glob1[:, 6:7], scalar2=glob1[:, 7:8],
                        op0=ALU.mult, op1=ALU.is_ge)
                    gpsimd.drain().then_inc(s_sg1, 1)
            gpsimd.wait_ge(dsem, D_B2DBIN)
            if CC_STUB:
                nc.gpsimd.dma_start(db2_out[:], db2_in[:]).then_inc(s_cc, 16)
            else:
                nc.gpsimd.collective_compute(
                    "AllReduce", ALU.add, replica_groups=[list(range(N_CORES))],
                    ins=[db2_in[:]], outs=[db2_out[:]]).then_inc(s_cc, 1)

    return nc


_CACHE = {}


def _get_nc():
    if "nc" not in _CACHE:
        _CACHE["nc"] = build_bass()
    return _CACHE["nc"]


def kernel(x, w1, gamma1, beta1, w2, gamma2, beta2):
    x = np.asarray(x, np.float32)
    w1 = np.asarray(w1, np.float32)
    w2 = np.asarray(w2, np.float32)
    gamma1 = np.asarray(gamma1, np.float32)
    beta1 = np.asarray(beta1, np.float32)
    gamma2 = np.asarray(gamma2, np.float32)
    beta2 = np.asarray(beta2, np.float32)

    # conv1 weights: [tap, cin, cout] -> [cin, tap*cout], rows duplicated
    wb1 = np.where(w1 >= 0, 1.0, -1.0).astype(np.float32)
    wt1 = wb1.transpose(1, 2, 3, 0).reshape(64, 9, 64).reshape(64, 576)
    wf16_np = np.concatenate([wt1, wt1], axis=0).astype(np.float16)

    # conv2 weights: sign(w2) as fp8, full-width DoubleRow with
    # block-diagonal planes (two images per matmul, two kh-taps per pass)
    wb2 = np.where(w2 >= 0, 1.0, -1.0).astype(np.float32)   # [o, i, kh, kw]
    wtap = wb2.transpose(2, 3, 1, 0)                        # [kh, kw, i, o]
    wfp8_np = np.zeros((128, NG2 * 256), np.float32)
    for g, (khA, kwA, zeroA) in enumerate(C2GROUPS):
        for i in (0, 1):
            if i == 0 and zeroA:
                continue
            blk = np.zeros((128, 128), np.float32)
            blk[0:64, 0:64] = wtap[khA + i, kwA]
            blk[64:128, 64:128] = wtap[khA + i, kwA]
            wfp8_np[:, g * 256 + i * 128 : g * 256 + (i + 1) * 128] = blk
    wfp8_np = wfp8_np.astype(ml_dtypes.float8_e4m3)

    S = wb2.sum(axis=(1, 2, 3))                             # [64] per out-ch
    consts_np = np.zeros((128, 8), np.float32)
    for col, v in enumerate([gamma1, beta1, gamma2, beta2, -0.5 * S]):
        consts_np[0:64, col] = v
        consts_np[64:128, col] = v

    in_maps = []
    for k in range(N_CORES):
        xc = x[IMGS * k : IMGS * (k + 1)]            # [8, 64, 56, 56]
        xp = np.zeros((IMGS, C, HP, HP), np.float32)
        xp[:, :, 1 : 1 + H, 1 : 1 + W] = xc
        arr = xp.reshape(SLOTS, 2, C, HP, HP).transpose(1, 2, 0, 3, 4)
        arr = np.ascontiguousarray(arr).reshape(128, SLOTS, HP, HP)
        ahi = arr.astype(np.float16)
        alo = (arr - ahi.astype(np.float32)).astype(np.float16)
        # second x copy in y1's permuted slot order (interior only) for
        # the final residual add
        xperm = np.empty((2, C, SLOTS, H, W), np.float32)
        for s in range(SLOTS):
            for h in (0, 1):
                xperm[h, :, s] = xc[IMG_OF[s][h]]
        x32_np = np.ascontiguousarray(xperm.reshape(128, YCOLS))
        in_maps.append({
            "xhi": ahi, "xlo": alo, "x32": x32_np,
            "wf16": wf16_np, "wfp8": wfp8_np, "consts": consts_np,
        })

    nc = _get_nc()
    res = bass_utils.run_bass_kernel_spmd(nc, in_maps, core_ids=list(range(N_CORES)))

    out = np.empty((N, C, H, W), np.float32)
    for k in range(N_CORES):
        o = np.asarray(res.results[k]["outp"]).astype(np.float32)  # [128, 12544]
        o = o.reshape(2, C, SLOTS, H, W)
        for s in range(SLOTS):
            for h in (0, 1):
                out[IMGS * k + IMG_OF[s][h]] = o[h, :, s]
    return out


if __name__ == "__main__":
    rng = np.random.default_rng(0)
    xs = rng.standard_normal((N, C, H, W)).astype(np.float32)
    w1s = (rng.standard_normal((C, C, 3, 3)) * 0.1).astype(np.float32)
    w2s = (rng.standard_normal((C, C, 3, 3)) * 0.1).astype(np.float32)
    ones = np.ones(C, np.float32)
    zeros = np.zeros(C, np.float32)
    r = kernel(x=xs, w1=w1s, gamma1=ones, beta1=zeros, w2=w2s, gamma2=ones,
               beta2=zeros)
    print("ran, out uniq:", np.unique(r))


# revision 47
# speedup vs baseline: 1.3346x; 1.0052x over previous
"""BinaryBasicBlock TRN2 kernel: 8-core batch-parallel, raw Bass.

Reference computation (per core: 8 images, C=64, 56x56):
  y1   = conv3x3(x, sign(w1))            # exact: x = fp16(x) + fp16(residual)
  bin1 = sign((y1 - mu1) * rsqrt(var1+eps) * g1 + b1)   # global batch stats
  y2   = conv3x3(bin1, sign(w2))         # exact
  out  = sign((y2 - mu2) * rsqrt(var2+eps) * g2 + b2 + x)

Batch stats are exact: per-core (sum, sumsq) partials are AllReduced across
the 8 cores mid-kernel.

v2 speedups over the baseline:
  - conv2 runs in fp8e4 with perf_mode=DoubleRow: bin1 is stored as
    {0,1} (0.5 at the padding halo) so +-1 inputs become exact fp8; the
    0/1 offset is folded into per-channel scalars via S_o = sum(sign(w2))
    (y2 = 2*y2' - S).  Taps pair along kh (pair step 64B, %16-aligned).
  - bin1 row pitch is 64 so a conv2 matmul streams one contiguous
    512-element window (8 rows x 64); the 8 junk columns per row are
    skipped at PSUM evacuation.
  - sign1 (bin1 = is_ge(a1*y1, -b1)) runs on the otherwise-idle GPSIMD
    engine, freeing ACT/DVE in the conv2 phase.
  - PSUM evacuation is split: ACT always reads the pbX banks, DVE always
    reads the pbY banks (one PSUM reader engine per bank).
  - sumsq for conv2 stats and both final residual passes run as all-f16
    tensor_scalar_ptr ops on DVE (4x DVE perf mode).

Toolchain constraints honored: raw Bass only, max one semaphore wait per
instruction, single PSUM reader engine per bank, drain-backed semaphore
increments on every cross-engine RAW edge, explicit DVE drains between
dependent vector ops.
"""
import numpy as np
import ml_dtypes
import concourse.bass as bass
import concourse.mybir as mybir
from concourse.ap import AP
from concourse import bass_utils
from contextlib import ExitStack

F32 = mybir.dt.float32
BF16 = mybir.dt.bfloat16
F16 = mybir.dt.float16
FP8 = mybir.dt.float8e4
AF = mybir.ActivationFunctionType
ALU = mybir.AluOpType
DR = mybir.MatmulPerfMode.DoubleRow

N_CORES = 8
N, C, H, W = 64, 64, 56, 56
IMGS = N // N_CORES          # 8 images per core
SLOTS = IMGS // 2            # 4 slots (2 images per slot)
QG = SLOTS // 2              # 2 quadgroups (4 images each)
HP = H + 2                   # 58 padded
BROWS = H + 3                # 59 rows in the fp8 bin1 (1 extra guard row)
BP = 64                      # bin1 row pitch
CHROWS = 8                   # output rows per 448-subchunk
CHUNK = CHROWS * W           # 448
BCHUNK = CHROWS * BP         # 512 (conv2 psum cols per subchunk)
NCH = H // CHROWS            # 7 subchunks per image
SUPERS = [(0, 2), (2, 4), (4, 6), (6, 7)]   # subchunk ranges per super-iter
NSUP = len(SUPERS)           # 4 super-iters per quadgroup
ITERS = QG * NSUP            # 8 super-iters per conv
PERIMG = H * W               # 3136
YCOLS = SLOTS * PERIMG       # 12544
N_TOT = float(N * H * W)     # global batch-stat count
EPS = 1e-5
NF = SLOTS * NSUP            # 16 final-stage iterations (per-slot supers)

# conv2 DoubleRow tap groups: plane A at (khA, kwA), plane B at (khA+1, kwA).
# zeroA marks groups whose A-plane weights are zero (kh=2 taps ride alone).
C2GROUPS = [
    (0, 0, False),   # taps (0,0)+(1,0)
    (0, 1, False),   # taps (0,1)+(1,1)
    (0, 2, False),   # taps (0,2)+(1,2)
    (1, 0, True),    # zero + tap (2,0)
    (1, 1, True),    # zero + tap (2,1)
    (1, 2, True),    # zero + tap (2,2)
]
NG2 = len(C2GROUPS)
NIT2 = SLOTS * NSUP          # 16 conv2 iterations (slot-major, full-width)

# y1/bin1/y2 slot layout after conv1's quad permutation: slot 2q holds
# images (4q, 4q+2) on its partition halves, slot 2q+1 holds (4q+1, 4q+3).
IMG_OF = {}
for _q in range(QG):
    IMG_OF[2 * _q] = (4 * _q, 4 * _q + 2)
    IMG_OF[2 * _q + 1] = (4 * _q + 1, 4 * _q + 3)

DEBUG = False
CC_STUB = False   # replace AllReduce with a local DMA (for TimelineSim)


def build_bass():
    nc = bass.Bass(trn_type="TRN2", target_bir_lowering=False, debug=False,
                   num_devices=N_CORES)

    d_xhi = nc.dram_tensor("xhi", [128, SLOTS, HP, HP], F16, kind="ExternalInput")
    d_xlo = nc.dram_tensor("xlo", [128, SLOTS, HP, HP], F16, kind="ExternalInput")
    d_x32 = nc.dram_tensor("x32", [128, YCOLS], F32, kind="ExternalInput")
    d_wf16 = nc.dram_tensor("wf16", [128, 576], F16, kind="ExternalInput")
    d_wfp8 = nc.dram_tensor("wfp8", [128, NG2 * 256], FP8, kind="ExternalInput")
    d_consts = nc.dram_tensor("consts", [128, 8], F32, kind="ExternalInput")
    d_out = nc.dram_tensor("outp", [128, YCOLS], BF16, kind="ExternalOutput")
    db1_in = nc.dram_tensor("db1_in", [128, 2], F32)
    db1_out = nc.dram_tensor("db1_out", [128, 2], F32, addr_space="Shared")
    db2_in = nc.dram_tensor("db2_in", [128, 2], F32)
    db2_out = nc.dram_tensor("db2_out", [128, 2], F32, addr_space="Shared")
    if DEBUG:
        d_g1 = nc.dram_tensor("dbg_g1", [128, 8], F32, kind="ExternalOutput")
        d_g2 = nc.dram_tensor("dbg_g2", [128, 8], F32, kind="ExternalOutput")
        d_y2 = nc.dram_tensor("dbg_y2", [128, YCOLS], F16, kind="ExternalOutput")

    es = ExitStack()
    def sb(name, shape, dt):
        return es.enter_context(nc.sbuf_tensor(name, shape, dt))
    def ps(name, shape, dt):
        return es.enter_context(nc.psum_tensor(name, shape, dt))
    def sem(name):
        return es.enter_context(nc.semaphore(name))

    xhi = sb("xhi_t", [128, SLOTS, HP, HP], F16)
    xlo = sb("xlo_t", [128, SLOTS, HP, HP], F16)
    x32 = sb("x32_t", [128, YCOLS], F32)
    wf16 = sb("wf16_t", [128, 576], F16)
    wfp8 = sb("wfp8_t", [128, NG2 * 256], FP8)
    consts = sb("consts_t", [128, 8], F32)
    bin1 = sb("bin1_t", [128, SLOTS, BROWS, BP], FP8)
    y1 = sb("y1_t", [128, YCOLS], F32)
    # y2 (fp16) and the output (bf16) live in y1's bytes (dead by then)
    y2v = y1[:].bitcast(F16)      # [128, 25088] f16 ; cols 0..12543 used
    outv = y1[:].bitcast(BF16)    # [128, 25088] bf16; cols 12544..25087 used
    OUTOFF = YCOLS
    sa1 = sb("sa1", [128, ITERS], F32)
    sb1 = sb("sb1", [128, ITERS], F32)
    qq1 = sb("qq1", [128, 2 * ITERS], F32)
    # conv2 evac op counts: ACT handles even iters, DVE odd iters
    EOPA = sum(SUPERS[it % NSUP][1] - SUPERS[it % NSUP][0]
               for it in range(NIT2) if it % 2 == 0)
    EOPB = sum(SUPERS[it % NSUP][1] - SUPERS[it % NSUP][0]
               for it in range(NIT2) if it % 2 == 1)
    sa2 = sb("sa2", [128, EOPA], F32)
    sb2 = sb("sb2", [128, EOPB], F32)
    qq2 = sb("qq2", [128, NIT2], F32)
    stats1 = sb("stats1", [128, 8], F32)
    stats2 = sb("stats2", [128, 8], F32)
    glob1 = sb("glob1", [128, 8], F32)
    glob2 = sb("glob2", [128, 8], F32)
    scr = [sb(f"scr{i}", [128, 2 * CHUNK], F32) for i in range(2)]
    scr16 = [s[:].bitcast(F16) for s in scr]
    wbuf = [sb(f"wb{i}", [128, 4 * CHUNK], F32) for i in range(2)]
    scrA = sb("scrA", [128, 2 * CHUNK], F32)
    # PSUM: 2 sets x (X, Y) tensors of 2 banks each = 8 banks
    pbX = [ps(f"pbX{i}", [128, 1024], F32) for i in range(2)]
    pbY = [ps(f"pbY{i}", [128, 1024], F32) for i in range(2)]

    dsem = sem("dsem")
    s_pe1 = sem("s_pe1"); s_pe2 = sem("s_pe2")
    s_eA1 = sem("s_eA1"); s_eB1 = sem("s_eB1")
    s_eA2 = sem("s_eA2"); s_eB2 = sem("s_eB2")
    s_sq1 = sem("s_sq1"); s_sq2 = sem("s_sq2")
    s_st1 = sem("s_st1"); s_st2 = sem("s_st2"); s_acst = sem("s_acst")
    s_m1 = sem("s_m1")
    s_sg1 = sem("s_sg1"); s_sgA = sem("s_sgA"); s_ms = sem("s_ms")
    s_qa = sem("s_qa")
    s_cc = sem("s_cc")
    s_fv = sem("s_fv"); s_fs = sem("s_fs")

    CCV = 16 if CC_STUB else 1
    # dsem milestones (each DMA increments by 16)
    D_QG0 = 4 * 16      # xhi01, xlo01, wf16, wfp8
    D_QG1 = 7 * 16      # xhi23, xlo23, consts
    D_XP = 9 * 16       # x32 halves
    D_B1DBIN = 10 * 16
    D_G1 = 13 * 16      # allreduce-1 result + swapped halves loaded
    D_B2DBIN = 14 * 16
    D_G2 = 17 * 16

    def ycol(slot, c):
        return slot * PERIMG + c * CHUNK

    # final-stage iteration table: (slot, sub0, nsub) — 8 bigger chunks
    FINALS = [(s, c0, c1 - c0) for s in range(SLOTS)
              for (c0, c1) in ((0, 4), (4, 7))]
    NFIN = len(FINALS)

    with nc.Block() as block:

        @block.sync
        def _(sync):
            sync.dma_start(xhi[:, 0:2], d_xhi[:, 0:2]).then_inc(dsem, 16)
            sync.dma_start(xlo[:, 0:2], d_xlo[:, 0:2]).then_inc(dsem, 16)
            sync.dma_start(wf16[:], d_wf16[:]).then_inc(dsem, 16)
            sync.dma_start(wfp8[:], d_wfp8[:]).then_inc(dsem, 16)
            sync.dma_start(xhi[:, 2:4], d_xhi[:, 2:4]).then_inc(dsem, 16)
            sync.dma_start(xlo[:, 2:4], d_xlo[:, 2:4]).then_inc(dsem, 16)
            sync.dma_start(consts[:], d_consts[:]).then_inc(dsem, 16)
            sync.dma_start(x32[:, 0 : YCOLS // 2],
                           d_x32[:, 0 : YCOLS // 2]).then_inc(dsem, 16)
            sync.dma_start(x32[:, YCOLS // 2 : YCOLS],
                           d_x32[:, YCOLS // 2 : YCOLS]).then_inc(dsem, 16)
            # stats1 chain: AllReduce the [128,2] partials, fold halves after
            sync.wait_ge(s_st1, 1)
            sync.dma_start(db1_in[:], stats1[:, 0:2]).then_inc(dsem, 16)
            sync.wait_ge(s_cc, CCV)
            sync.dma_start(glob1[:, 0:2], db1_out[:]).then_inc(dsem, 16)
            sync.dma_start(glob1[0:64, 2:4], db1_out[64:128]).then_inc(dsem, 16)
            sync.dma_start(glob1[64:128, 2:4], db1_out[0:64]).then_inc(dsem, 16)
            # stats2 chain
            sync.wait_ge(s_st2, 1)
            sync.dma_start(db2_in[:], stats2[:, 0:2]).then_inc(dsem, 16)
            sync.wait_ge(s_cc, 2 * CCV)
            sync.dma_start(glob2[:, 0:2], db2_out[:]).then_inc(dsem, 16)
            sync.dma_start(glob2[0:64, 2:4], db2_out[64:128]).then_inc(dsem, 16)
            sync.dma_start(glob2[64:128, 2:4], db2_out[0:64]).then_inc(dsem, 16)
            # output stores (one per slot)
            for s in range(SLOTS):
                sync.wait_ge(s_fs, 2 * (s + 1))
                sync.dma_start(
                    d_out[:, s * PERIMG : (s + 1) * PERIMG],
                    outv[:, OUTOFF + s * PERIMG : OUTOFF + (s + 1) * PERIMG]
                ).then_inc(dsem, 16)
            if DEBUG:
                sync.dma_start(d_g1[:], glob1[:]).then_inc(dsem, 16)
                sync.dma_start(d_g2[:], glob2[:]).then_inc(dsem, 16)
                sync.dma_start(d_y2[:], y2v[:, 0:YCOLS]).then_inc(dsem, 16)

        @block.tensor
        def _(tensor):
            # conv1: f16, 9 taps, two passes (hi + lo) into the same psum
            it = 0
            for q in range(QG):
                tensor.wait_ge(dsem, (D_QG0, D_QG1)[q])
                for (c0, c1) in SUPERS:
                    nsub = c1 - c0
                    if it >= 2:
                        tensor.wait_ge(s_eA1, it - 1)
                        tensor.wait_ge(s_eB1, it - 1)
                    pX = pbX[it % 2]
                    pY = pbY[it % 2]
                    quads = [
                        ((0, 0), slice(0, 64), 2 * q, pX, slice(0, 64)),
                        ((64, 0), slice(64, 128), 2 * q, pY, slice(0, 64)),
                        ((0, 64), slice(0, 64), 2 * q + 1, pX, slice(64, 128)),
                        ((64, 64), slice(64, 128), 2 * q + 1, pY,
                         slice(64, 128)),
                    ]
                    for tap in range(9):
                        kh, kw = tap // 3, tap % 3
                        wcol = tap * 64
                        for tp, rows, _, _, _ in quads:
                            nc.tensor.ldweights(wf16[rows, wcol : wcol + 64],
                                                tile_position=tp)
                        for ip, rhs_t in enumerate([xhi, xlo]):
                            for tp, rows, dslot, pdst, phalf in quads:
                                for s in range(nsub):
                                    c = c0 + s
                                    first = ip == 0 and tap == 0
                                    last = ip == 1 and tap == 8
                                    rap = rhs_t[rows, dslot,
                                                c * CHROWS + kh :
                                                c * CHROWS + kh + CHROWS,
                                                kw : kw + W]
                                    nc.tensor.matmul(
                                        pdst[phalf, s * 512 : s * 512 + CHUNK],
                                        wf16[rows, wcol : wcol + 64], rap,
                                        start=first, stop=last,
                                        tile_position=tp,
                                        skip_group_check=True)
                    tensor.drain().then_inc(s_pe1, 1)
                    it += 1

            # conv2: full-width fp8 DoubleRow (block-diagonal weights handle
            # both images of a slot per matmul), slot-major, 16 iterations
            PSUMS = [pbX[0], pbY[0], pbX[1], pbY[1]]
            wg0 = wfp8[:, 0:256].rearrange("p (a b) -> p a b", a=2)
            nc.tensor.ldweights(wg0, perf_mode=DR)
            for it in range(NIT2):
                slot, ci = it // NSUP, it % NSUP
                c0, c1 = SUPERS[ci]
                nsub = c1 - c0
                tensor.wait_ge(s_sg1, it + 1)
                if it >= 4:
                    if it % 2 == 0:
                        tensor.wait_ge(s_eA2, (it - 4) // 2 + 1)
                    else:
                        tensor.wait_ge(s_eB2, (it - 5) // 2 + 1)
                pdst = PSUMS[it % 4]
                v = bin1[:, slot]
                pstride = v.ap[0][0]
                for g, (khA, kwA, _) in enumerate(C2GROUPS):
                    wg = wfp8[:, g * 256 : (g + 1) * 256].rearrange(
                        "p (a b) -> p a b", a=2)
                    nc.tensor.ldweights(wg, perf_mode=DR)
                    for s in range(nsub):
                        c = c0 + s
                        off = v.offset + (c * CHROWS + khA) * BP + kwA
                        rap = AP(tensor=v.tensor, offset=off,
                                 ap=[[pstride, 128], [BP, 2], [1, BCHUNK]])
                        nc.tensor.matmul(
                            pdst[:, s * 512 : (s + 1) * 512],
                            wg, rap,
                            start=(g == 0), stop=(g == NG2 - 1),
                            perf_mode=DR,
                            skip_group_check=True)
                tensor.drain().then_inc(s_pe2, 1)

        @block.scalar
        def _(scalar):
            # conv1 evac: ACT reads pbX (slots 2q), sum accum into sa1
            it = 0
            for q in range(QG):
                for (c0, c1) in SUPERS:
                    nsub = c1 - c0
                    scalar.wait_ge(s_pe1, it + 1)
                    pX = pbX[it % 2]
                    src = pX[:, 0 : nsub * 512].rearrange(
                        "p (s k) -> p s k", s=nsub)[:, :, 0:CHUNK]
                    nc.scalar.activation(
                        y1[:, ycol(2 * q, c0) : ycol(2 * q, c0) + nsub * CHUNK],
                        src, AF.Copy,
                        accum_out=sa1[:, it : it + 1])
                    scalar.drain().then_inc(s_eA1, 1)
                    it += 1
            # ACT picks up the last super's slot-2 sumsq (slot 3 stays on
            # DVE, fresh from its own evac) so neither tail is long
            yc = y1[:, ycol(2, 6) : ycol(2, 6) + CHUNK]
            nc.scalar.activation(
                scrA[:, 0:CHUNK], yc, AF.Square,
                accum_out=qq1[:, 14:15])
            scalar.drain().then_inc(s_qa, 1)
            # stats1: sqrt(var + eps)
            scalar.wait_ge(s_st1, 2)
            nc.scalar.activation(glob1[:, 4:5], glob1[:, 5:6], AF.Sqrt)
            scalar.drain().then_inc(s_acst, 1)
            # conv2 evac: ACT reads the pbX-rotation iters (even its),
            # per-subchunk so the 8 junk cols per row are skipped
            PSUMS_S = [pbX[0], pbY[0], pbX[1], pbY[1]]
            eop = 0
            for it in range(NIT2):
                if it % 2 != 0:
                    continue
                slot, ci = it // NSUP, it % NSUP
                c0, c1 = SUPERS[ci]
                nsub = c1 - c0
                scalar.wait_ge(s_pe2, it + 1)
                pt = PSUMS_S[it % 4][:]
                pstride = pt.ap[0][0]
                for s in range(nsub):
                    src = AP(tensor=pt.tensor, offset=pt.offset + s * 512,
                             ap=[[pstride, 128], [BP, CHROWS], [1, W]])
                    # y2'' = y2' - S/2, so y2 = 2*y2'' exactly (S is even)
                    nc.scalar.activation(
                        y2v[:, ycol(slot, c0 + s) :
                            ycol(slot, c0 + s) + CHUNK],
                        src, AF.Identity, bias=consts[:, 4:5],
                        accum_out=sa2[:, eop : eop + 1])
                    eop += 1
                scalar.drain().then_inc(s_eA2, 1)
                # interleave the trailing sumsq squares (iters 12-14) right
                # after their data is available, before the next evac
                if it == 12:
                    sq_its = (12, 13)
                elif it == 14:
                    sq_its = (14,)
                else:
                    sq_its = ()
                for sit in sq_its:
                    sslot, sci = sit // NSUP, sit % NSUP
                    sc0, sc1 = SUPERS[sci]
                    if sit % 2 == 1:
                        scalar.wait_ge(s_eB2, (sit + 1) // 2)
                    yc = y2v[:, ycol(sslot, sc0) :
                             ycol(sslot, sc0) + (sc1 - sc0) * CHUNK]
                    nc.scalar.activation(
                        scrA[:, 0 : (sc1 - sc0) * CHUNK], yc, AF.Square,
                        scale=0.125, accum_out=qq2[:, sit : sit + 1])
                if it == 14:
                    scalar.drain().then_inc(s_qa, 2)
            # stats2 sqrt
            scalar.wait_ge(s_st2, 2)
            nc.scalar.activation(glob2[:, 4:5], glob2[:, 5:6], AF.Sqrt)
            scalar.drain().then_inc(s_acst, 2)
            # final: sign2 = Sign(w + bias2')
            for j in range(NFIN):
                sl, c0, nsub = FINALS[j]
                scalar.wait_ge(s_fv, j + 1)
                nc.scalar.activation(
                    outv[:, OUTOFF + ycol(sl, c0) :
                         OUTOFF + ycol(sl, c0) + nsub * CHUNK],
                    wbuf[j % 2][:, 0 : nsub * CHUNK], AF.Sign,
                    bias=glob2[:, 7:8])
                scalar.drain().then_inc(s_fs, 1)

        @block.vector
        def _(vector):
            # conv1: DVE evacs pbY (slots 2q+1) + sumsq over both slots
            it = 0
            for q in range(QG):
                for (c0, c1) in SUPERS:
                    nsub = c1 - c0
                    vector.wait_ge(s_pe1, it + 1)
                    pY = pbY[it % 2]
                    src = pY[:, 0 : nsub * 512].rearrange(
                        "p (s k) -> p s k", s=nsub)[:, :, 0:CHUNK]
                    nc.vector.tensor_scalar(
                        out=y1[:, ycol(2 * q + 1, c0) :
                               ycol(2 * q + 1, c0) + nsub * CHUNK],
                        in0=src, scalar1=0.0, scalar2=None,
                        op0=ALU.add, op1=ALU.add,
                        accum_out=sb1[:, it : it + 1])
                    nc.vector.drain().then_inc(s_eB1, 1)
                    if it < 7:
                        vector.wait_ge(s_eA1, it + 1)
                        pairs = ((0, 2 * q), (1, 2 * q + 1))
                    else:
                        pairs = ((1, 3),)   # slot 3 only; ACT covers slot 2
                    for half, slot in pairs:
                        yc = y1[:, ycol(slot, c0) :
                                ycol(slot, c0) + nsub * CHUNK]
                        nc.vector.scalar_tensor_tensor(
                            out=scr[it % 2][:, 0 : nsub * CHUNK], in0=yc,
                            scalar=1.0, in1=yc,
                            op0=ALU.mult, op1=ALU.mult,
                            accum_out=qq1[:, 2 * it + half :
                                          2 * it + half + 1])
                    nc.vector.drain()
                    it += 1

            # stats1 fold + math: a1 = g1*rsqrt(var+eps), nb1 = m*a1 - b1
            vector.wait_ge(s_qa, 1)
            nc.vector.reduce_sum(stats1[:, 6:7], sa1[:], axis=mybir.AxisListType.X)
            nc.vector.reduce_sum(stats1[:, 7:8], sb1[:], axis=mybir.AxisListType.X)
            nc.vector.reduce_sum(stats1[:, 1:2], qq1[:], axis=mybir.AxisListType.X)
            nc.vector.drain()
            nc.vector.tensor_tensor(out=stats1[:, 0:1], in0=stats1[:, 6:7],
                                    in1=stats1[:, 7:8], op=ALU.add)
            nc.vector.drain().then_inc(s_st1, 1)
            vector.wait_ge(dsem, D_G1)
            nc.vector.tensor_tensor(out=glob1[:, 0:2], in0=glob1[:, 0:2],
                                    in1=glob1[:, 2:4], op=ALU.add)
            nc.vector.drain()
            nc.vector.tensor_scalar_mul(glob1[:, 2:4], glob1[:, 0:2],
                                        1.0 / N_TOT)
            nc.vector.drain()
            nc.vector.tensor_tensor(out=glob1[:, 4:5], in0=glob1[:, 2:3],
                                    in1=glob1[:, 2:3], op=ALU.mult)
            nc.vector.drain()
            nc.vector.tensor_tensor(out=glob1[:, 5:6], in0=glob1[:, 3:4],
                                    in1=glob1[:, 4:5], op=ALU.subtract)
            nc.vector.drain()
            nc.vector.tensor_scalar_add(glob1[:, 5:6], glob1[:, 5:6], EPS)
            nc.vector.drain().then_inc(s_st1, 1)
            vector.wait_ge(s_acst, 1)
            nc.vector.reciprocal(glob1[:, 3:4], glob1[:, 4:5])
            nc.vector.drain()
            nc.vector.tensor_tensor(out=glob1[:, 6:7], in0=glob1[:, 3:4],
                                    in1=consts[:, 0:1], op=ALU.mult)
            nc.vector.drain()
            nc.vector.tensor_tensor(out=glob1[:, 4:5], in0=glob1[:, 2:3],
                                    in1=glob1[:, 6:7], op=ALU.mult)
            nc.vector.drain()
            nc.vector.tensor_tensor(out=glob1[:, 7:8], in0=glob1[:, 4:5],
                                    in1=consts[:, 1:2], op=ALU.subtract)
            nc.vector.drain().then_inc(s_m1, 1)

            # conv2: DVE evacs the pbY-rotation iters (odd its) + f16 sumsq
            # over every iter's fresh y2' columns (scaled by 1/64)
            PSUMS_V = [pbX[0], pbY[0], pbX[1], pbY[1]]
            eop = 0
            for it in range(NIT2):
                slot, ci = it // NSUP, it % NSUP
                c0, c1 = SUPERS[ci]
                nsub = c1 - c0
                if it % 2 == 1:
                    vector.wait_ge(s_pe2, it + 1)
                    pt = PSUMS_V[it % 4][:]
                    pstride = pt.ap[0][0]
                    for s in range(nsub):
                        src = AP(tensor=pt.tensor, offset=pt.offset + s * 512,
                                 ap=[[pstride, 128], [BP, CHROWS], [1, W]])
                        nc.vector.tensor_scalar(
                            out=y2v[:, ycol(slot, c0 + s) :
                                    ycol(slot, c0 + s) + CHUNK],
                            in0=src, scalar1=consts[:, 4:5], scalar2=None,
                            op0=ALU.add, op1=ALU.add,
                            accum_out=sb2[:, eop : eop + 1])
                        eop += 1
                    nc.vector.drain().then_inc(s_eB2, 1)
                elif it < 12:
                    vector.wait_ge(s_eA2, it // 2 + 1)
                if it < 12 or it == 15:
                    yc = y2v[:, ycol(slot, c0) : ycol(slot, c0) + nsub * CHUNK]
                    nc.vector.scalar_tensor_tensor(
                        out=scr16[it % 2][:, 0 : nsub * CHUNK], in0=yc,
                        scalar=1.0 / 64.0, in1=yc,
                        op0=ALU.mult, op1=ALU.mult,
                        accum_out=qq2[:, it : it + 1])
                    nc.vector.drain()

            # stats2 fold + math: y2 = 2*y2'' exactly, so
            #   m2 = 2*m'' ; var2 = 256*q'' - (2*m'')^2 ; SC = 2*g2*rsqrt(var2+eps)
            vector.wait_ge(s_qa, 2)
            nc.vector.reduce_sum(stats2[:, 6:7], sa2[:], axis=mybir.AxisListType.X)
            nc.vector.reduce_sum(stats2[:, 7:8], sb2[:], axis=mybir.AxisListType.X)
            nc.vector.reduce_sum(stats2[:, 1:2], qq2[:], axis=mybir.AxisListType.X)
            nc.vector.drain()
            nc.vector.tensor_tensor(out=stats2[:, 0:1], in0=stats2[:, 6:7],
                                    in1=stats2[:, 7:8], op=ALU.add)
            nc.vector.drain().then_inc(s_st2, 1)
            vector.wait_ge(dsem, D_G2)
            nc.vector.tensor_tensor(out=glob2[:, 0:2], in0=glob2[:, 0:2],
                                    in1=glob2[:, 2:4], op=ALU.add)
            nc.vector.drain()
            nc.vector.tensor_scalar_mul(glob2[:, 2:4], glob2[:, 0:2],
                                        1.0 / N_TOT)
            nc.vector.drain()
            # col4 = (2*m'')^2 ; col5 = 256*q'' - col4 + eps = var2 + eps
            nc.vector.tensor_scalar_mul(glob2[:, 4:5], glob2[:, 2:3], 2.0)
            nc.vector.drain()
            nc.vector.tensor_tensor(out=glob2[:, 4:5], in0=glob2[:, 4:5],
                                    in1=glob2[:, 4:5], op=ALU.mult)
            nc.vector.drain()
            nc.vector.scalar_tensor_tensor(
                out=glob2[:, 5:6], in0=glob2[:, 3:4], scalar=256.0,
                in1=glob2[:, 4:5], op0=ALU.mult, op1=ALU.subtract)
            nc.vector.drain()
            nc.vector.tensor_scalar_add(glob2[:, 5:6], glob2[:, 5:6], EPS)
            nc.vector.drain().then_inc(s_st2, 1)
            vector.wait_ge(s_acst, 2)
            nc.vector.reciprocal(glob2[:, 3:4], glob2[:, 4:5])
            nc.vector.drain()
            # col6 = A2 = recip * g2 ; col7 = bias2' = beta2 - 2*A2*m'' ;
            # then col6 = SC = 2*A2  (m'' still lives in col2)
            nc.vector.tensor_tensor(out=glob2[:, 6:7], in0=glob2[:, 3:4],
                                    in1=consts[:, 2:3], op=ALU.mult)
            nc.vector.drain()
            nc.vector.tensor_tensor(out=glob2[:, 4:5], in0=glob2[:, 6:7],
                                    in1=glob2[:, 2:3], op=ALU.mult)
            nc.vector.drain()
            nc.vector.tensor_scalar_mul(glob2[:, 4:5], glob2[:, 4:5], 2.0)
            nc.vector.drain()
            nc.vector.tensor_tensor(out=glob2[:, 7:8], in0=consts[:, 3:4],
                                    in1=glob2[:, 4:5], op=ALU.subtract)
            nc.vector.tensor_scalar_mul(glob2[:, 6:7], glob2[:, 6:7], 2.0)
            nc.vector.drain()

            # final: w = SC*y2'' + x32 in f32 (single pass; bias2'
            # is applied inside the ACT Sign at f32 precision)
            for j in range(NFIN):
                sl, c0, nsub = FINALS[j]
                cols = slice(ycol(sl, c0), ycol(sl, c0) + nsub * CHUNK)
                if j >= 2:
                    vector.wait_ge(s_fs, j - 1)
                nc.vector.scalar_tensor_tensor(
                    out=wbuf[j % 2][:, 0 : nsub * CHUNK],
                    in0=y2v[:, cols],
                    scalar=glob2[:, 6:7],
                    in1=x32[:, cols],
                    op0=ALU.mult, op1=ALU.add)
                nc.vector.drain().then_inc(s_fv, 1)

        @block.gpsimd
        def _(gpsimd):
            # bin1 halo = 0.5 (== (0+1)/2, the zero-pad in {0,1} space)
            for s in range(SLOTS):
                nc.gpsimd.memset(bin1[:, s], 0.5)
            gpsimd.drain()
            gpsimd.wait_ge(dsem, D_B1DBIN)
            if CC_STUB:
                nc.gpsimd.dma_start(db1_out[:], db1_in[:]).then_inc(s_cc, 16)
            else:
                nc.gpsimd.collective_compute(
                    "AllReduce", ALU.add, replica_groups=[list(range(N_CORES))],
                    ins=[db1_in[:]], outs=[db1_out[:]]).then_inc(s_cc, 1)
            # sign1: bin1 = (a1*y1 >= -b1) in {0,1}, written as fp8.
            # Slot-major emission matches conv2's iteration order.
            gpsimd.wait_ge(s_m1, 1)
            for slot in range(SLOTS):
                for (c0, c1) in SUPERS:
                    nsub = c1 - c0
                    nc.gpsimd.tensor_scalar(
                        out=bin1[:, slot, 1 + c0 * CHROWS :
                                 1 + c1 * CHROWS, 1 : 1 + W],
                        in0=y1[:, ycol(slot, c0) :
                               ycol(slot, c0) + nsub * CHUNK],
                        scalar1=glob1[:, 6:7], scalar2=glob1[:, 7:8],
                        op0=ALU.mult, op1=ALU.is_ge)
                    gpsimd.drain().then_inc(s_sg1, 1)
            gpsimd.wait_ge(dsem, D_B2DBIN)
            if CC_STUB:
                nc.gpsimd.dma_start(db2_out[:], db2_in[:]).then_inc(s_cc, 16)
            else:
                nc.gpsimd.collective_compute(
                    "AllReduce", ALU.add, replica_groups=[list(range(N_CORES))],
                    ins=[db2_in[:]], outs=[db2_out[:]]).then_inc(s_cc, 1)

    return nc


_CACHE = {}


def _get_nc():
    if "nc" not in _CACHE:
        _CACHE["nc"] = build_bass()
    return _CACHE["nc"]


def kernel(x, w1, gamma1, beta1, w2, gamma2, beta2):
    x = np.asarray(x, np.float32)
    w1 = np.asarray(w1, np.float32)
    w2 = np.asarray(w2, np.float32)
    gamma1 = np.asarray(gamma1, np.float32)
    beta1 = np.asarray(beta1, np.float32)
    gamma2 = np.asarray(gamma2, np.float32)
    beta2 = np.asarray(beta2, np.float32)

    # conv1 weights: [tap, cin, cout] -> [cin, tap*cout], rows duplicated
    wb1 = np.where(w1 >= 0, 1.0, -1.0).astype(np.float32)
    wt1 = wb1.transpose(1, 2, 3, 0).reshape(64, 9, 64).reshape(64, 576)
    wf16_np = np.concatenate([wt1, wt1], axis=0).astype(np.float16)

    # conv2 weights: sign(w2) as fp8, full-width DoubleRow with
    # block-diagonal planes (two images per matmul, two kh-taps per pass)
    wb2 = np.where(w2 >= 0, 1.0, -1.0).astype(np.float32)   # [o, i, kh, kw]
    wtap = wb2.transpose(2, 3, 1, 0)                        # [kh, kw, i, o]
    wfp8_np = np.zeros((128, NG2 * 256), np.float32)
    for g, (khA, kwA, zeroA) in enumerate(C2GROUPS):
        for i in (0, 1):
            if i == 0 and zeroA:
                continue
            blk = np.zeros((128, 128), np.float32)
            blk[0:64, 0:64] = wtap[khA + i, kwA]
            blk[64:128, 64:128] = wtap[khA + i, kwA]
            wfp8_np[:, g * 256 + i * 128 : g * 256 + (i + 1) * 128] = blk
    wfp8_np = wfp8_np.astype(ml_dtypes.float8_e4m3)

    S = wb2.sum(axis=(1, 2, 3))                             # [64] per out-ch
    consts_np = np.zeros((128, 8), np.float32)
    for col, v in enumerate([gamma1, beta1, gamma2, beta2, -0.5 * S]):
        consts_np[0:64, col] = v
        consts_np[64:128, col] = v

    in_maps = []
    for k in range(N_CORES):
        xc = x[IMGS * k : IMGS * (k + 1)]            # [8, 64, 56, 56]
        xp = np.zeros((IMGS, C, HP, HP), np.float32)
        xp[:, :, 1 : 1 + H, 1 : 1 + W] = xc
        arr = xp.reshape(SLOTS, 2, C, HP, HP).transpose(1, 2, 0, 3, 4)
        arr = np.ascontiguousarray(arr).reshape(128, SLOTS, HP, HP)
        ahi = arr.astype(np.float16)
        alo = (arr - ahi.astype(np.float32)).astype(np.float16)
        # second x copy in y1's permuted slot order (interior only) for
        # the final residual add
        xperm = np.empty((2, C, SLOTS, H, W), np.float32)
        for s in range(SLOTS):
            for h in (0, 1):
                xperm[h, :, s] = xc[IMG_OF[s][h]]
        x32_np = np.ascontiguousarray(xperm.reshape(128, YCOLS))
        in_maps.append({
            "xhi": ahi, "xlo": alo, "x32": x32_np,
            "wf16": wf16_np, "wfp8": wfp8_np, "consts": consts_np,
        })

    nc = _get_nc()
    res = bass_utils.run_bass_kernel_spmd(nc, in_maps, core_ids=list(range(N_CORES)))

    out = np.empty((N, C, H, W), np.float32)
    for k in range(N_CORES):
        o = np.asarray(res.results[k]["outp"]).astype(np.float32)  # [128, 12544]
        o = o.reshape(2, C, SLOTS, H, W)
        for s in range(SLOTS):
            for h in (0, 1):
                out[IMGS * k + IMG_OF[s][h]] = o[h, :, s]
    return out


if __name__ == "__main__":
    rng = np.random.default_rng(0)
    xs = rng.standard_normal((N, C, H, W)).astype(np.float32)
    w1s = (rng.standard_normal((C, C, 3, 3)) * 0.1).astype(np.float32)
    w2s = (rng.standard_normal((C, C, 3, 3)) * 0.1).astype(np.float32)
    ones = np.ones(C, np.float32)
    zeros = np.zeros(C, np.float32)
    r = kernel(x=xs, w1=w1s, gamma1=ones, beta1=zeros, w2=w2s, gamma2=ones,
               beta2=zeros)
    print("ran, out uniq:", np.unique(r))
